# revision 26
# baseline (speedup 1.0000x reference)
"""Trainium2 Bass kernel for nn_ExchangeableLayer (segment_reduce).

out[e] = relu( x[e] @ th00
             + (segmean(t0, cols) @ th10)[c_e]
             + (segmean(t0, rows) @ th01)[r_e]
             + (segmean(t1, t1cols) @ th1x0_10)[c_e]
             + (segmean(t2, t2rows) @ th2x0_01)[r_e]
             + mean(t0) @ th11 + mean(t1) @ th1x0_11 + mean(t2) @ th2x0_11
             + theta_b )

Strategy: sort entries by segment id on host, shard contiguously by segment
range across 8 cores.  Per core:
  A) segment sums via PE one-hot matmuls into per-128-segment PSUM windows
     (tables kept transposed [64, segs] in SBUF); one-hots built in ONE
     batched DVE op per window.
  B) scale by host-precomputed 1/(cnt+eps), apply thetas (PE), fold the
     global-mean term into the col table; col table stays resident in SBUF
     [128seg, NWc, 64]; row table cast to bf16 rows padded to 128 elems and
     AllGathered to a full [8*NP, 128] bf16 DRAM table.
  C) per-entry phase, TRANSPOSED layout [64, entries], col-sorted order:
       z   = th00^T @ xT            (one stationary load, streams entries)
       ct  = ct_win^T @ ohT         (accumulated into same PSUM)
       ohT built via K=1 ones-matmul replication of rel + DVE is_equal
       rt  = dma_gather(transpose=True) of bf16 row-table rows; rr-queued
             across 4 SWDGE queues
       out = relu(psum + rt) -> [64, E] f32 -> DRAM; host un-transposes.
"""

import math
import os
import sys
import types

import numpy as np

for _p in ("/root/.axon_site/_ro/trn_rl_repo", "/opt/trn_rl_repo"):
    if os.path.isdir(_p) and _p not in sys.path:
        sys.path.append(_p)

import ml_dtypes

import concourse.bass as bass
import concourse.mybir as mybir
from concourse import bacc, tile
from concourse.bass_utils import run_bass_kernel_spmd

BF16 = ml_dtypes.bfloat16
F32 = np.float32
NCORES = 8
U = 64
WIN = 128
EPS = 1e-10
GM = 4096          # phase-C gather/stream macro (entries)
CH = 512           # phase-C psum chunk (entries)
IDX_PAD = 128      # trailing non-negative idxs per gather call

# Full-size problem dims (the graded problem).
FULL_DIMS = dict(N=50000, M=10000, NNZ0=1_000_000, NNZ1=500_000, NNZ2=500_000)


# --------------------------------------------------------------------------
# host-side preparation
# --------------------------------------------------------------------------

def _prep_stream(ids, seg_sl):
    """Sort entries by id, shard contiguously at multiples of seg_sl."""
    order = np.argsort(ids, kind="stable").astype(np.int64)
    sids = ids[order]
    bounds = np.searchsorted(sids, seg_sl * np.arange(NCORES + 1)).astype(np.int64)
    NW = -(-seg_sl // WIN)
    cores = []
    kmax = 1
    for c in range(NCORES):
        lo, hi = int(bounds[c]), int(bounds[c + 1])
        clen = hi - lo
        loc = (sids[lo:hi] - seg_sl * c).astype(np.int64)
        tc = -(-clen // 128)
        ws = np.searchsorted(loc, WIN * np.arange(NW + 1))
        wt = []
        for w in range(NW):
            a, b = int(ws[w]), int(ws[w + 1])
            if b > a:
                t0, t1 = a // 128, (b - 1) // 128
                wt.append((t0, t1 - t0 + 1))
                kmax = max(kmax, t1 - t0 + 1)
            else:
                wt.append((0, 0))
        cores.append(dict(clen=clen, loc=loc, corder=order[lo:hi], tc=tc, wt=wt))
    return dict(NW=NW, kmax=kmax, cores=cores)


def _mat_stream(stream, S, nnz):
    """Materialize per-core slot arrays: entry indices + rel ids."""
    NW, K = stream["NW"], stream["kmax"]
    for core in stream["cores"]:
        idx = np.full((S, 128), nnz, np.int64)
        rel = np.full((S, 128), -1.0, np.float32)
        tc, clen = core["tc"], core["clen"]
        locp = np.full(tc * 128, -(10 ** 6), np.int64)
        locp[:clen] = core["loc"]
        cordp = np.full(tc * 128, nnz, np.int64)
        cordp[:clen] = core["corder"]
        first_slot = np.full(max(tc, 1), -1, np.int64)
        for w, (t0, nt) in enumerate(core["wt"]):
            for k in range(nt):
                t = t0 + k
                s = w * K + k
                idx[s] = cordp[t * 128:(t + 1) * 128]
                rel[s] = locp[t * 128:(t + 1) * 128] - WIN * w
                if first_slot[t] < 0:
                    first_slot[t] = s
        core["idx"] = idx
        core["rel"] = rel
        core["first_slot"] = first_slot


def _prepare(inputs, dims):
    """All host-side metadata + per-core input arrays."""
    N, M = dims["N"], dims["M"]
    NNZ0, NNZ1, NNZ2 = dims["NNZ0"], dims["NNZ1"], dims["NNZ2"]
    M_SL, N_SL = M // NCORES, N // NCORES

    t0_rows = np.asarray(inputs["t0_rows"], np.int64)
    t0_cols = np.asarray(inputs["t0_cols"], np.int64)
    t1_cols = np.asarray(inputs["t1_cols"], np.int64)
    t2_rows = np.asarray(inputs["t2_rows"], np.int64)

    st0c = _prep_stream(t0_cols, M_SL)
    st0r = _prep_stream(t0_rows, N_SL)
    st1c = _prep_stream(t1_cols, M_SL)
    st2r = _prep_stream(t2_rows, N_SL)

    # uniform slot counts; pad S0c to a multiple of GM/128 for phase C macros
    SLOTS_PER_GM = GM // 128
    S0c = -(-(st0c["NW"] * st0c["kmax"]) // SLOTS_PER_GM) * SLOTS_PER_GM
    S0r = st0r["NW"] * st0r["kmax"]
    S1c = st1c["NW"] * st1c["kmax"]
    S2r = st2r["NW"] * st2r["kmax"]

    _mat_stream(st0c, S0c, NNZ0)
    _mat_stream(st0r, S0r, NNZ0)
    _mat_stream(st1c, S1c, NNZ1)
    _mat_stream(st2r, S2r, NNZ2)

    NWc, NWr = st0c["NW"], st0r["NW"]
    MP, NP = NWc * 128, NWr * 128          # padded per-core table slice rows
    E = S0c * 128                           # phase-C padded entries per core
    NG = E // GM                            # gather macros

    x0 = np.asarray(inputs["t0_values"], np.float32)
    x1 = np.asarray(inputs["t1_values"], np.float32)
    x2 = np.asarray(inputs["t2_values"], np.float32)
    x0e = np.concatenate([x0, np.zeros((1, U), np.float32)]).astype(BF16)
    x1e = np.concatenate([x1, np.zeros((1, U), np.float32)]).astype(BF16)
    x2e = np.concatenate([x2, np.zeros((1, U), np.float32)]).astype(BF16)

    def _inv(ids, nseg):
        cnt = np.bincount(ids, minlength=nseg).astype(np.float32)
        return (1.0 / (cnt + np.float32(EPS))).astype(np.float32)

    inv_c0 = _inv(t0_cols, M)
    inv_r0 = _inv(t0_rows, N)
    inv_c1 = _inv(t1_cols, M)
    inv_r2 = _inv(t2_rows, N)

    def _slice_pad(arr, sl, pad_to):
        out = np.ones(pad_to, np.float32)
        out[: sl.stop - sl.start] = arr[sl]
        return out

    rext = np.concatenate([t0_rows, [0]])

    # shared constants
    iota_b = np.broadcast_to(np.arange(128, dtype=np.float32), (128, 128)).astype(BF16)
    iota_p = np.arange(128, dtype=np.float32).reshape(128, 1).astype(BF16)
    ident_f = np.eye(128, dtype=np.float32)
    ident_b = np.eye(128, dtype=np.float32).astype(BF16)
    ones_f = np.ones((1, U), np.float32)
    ones_row = np.ones((1, 128), np.float32).astype(BF16)
    th = {k: np.asarray(inputs[k], np.float32) for k in
          ("theta_00", "theta_10", "theta_01", "theta_11", "theta_1x0_10",
           "theta_1x0_11", "theta_2x0_01", "theta_2x0_11")}
    th00b = th["theta_00"].astype(BF16)      # [64, 64] lhsT for z
    thbT = np.asarray(inputs["theta_b"], np.float32).reshape(U, 1)

    # rt table geometry: full table rows = NCORES * NP (row-padded bf16[128])
    RT_ROWS = NCORES * NP
    BIAS = 32767 if RT_ROWS > 32767 else 0

    in_maps = []
    post = []
    for c in range(NCORES):
        c0, r0, c1, r2 = (st0c["cores"][c], st0r["cores"][c],
                          st1c["cores"][c], st2r["cores"][c])
        # phase-C x stream, transposed: [64, E] bf16 (col-sorted slot order)
        x0c_a = x0e[c0["idx"]]                      # [S0c, 128, 64] bf16
        xT = np.ascontiguousarray(
            x0c_a.transpose(2, 0, 1).reshape(U, E))

        # phase-C rel stream [1, E] bf16 (col rel within window; pad = -1)
        relC = np.ascontiguousarray(c0["rel"].reshape(1, E)).astype(BF16)

        # phase-C rt gather indices: global row id -> padded table position
        rr = rext[c0["idx"]].reshape(E)             # [E]
        rpos_flat = (NP * (rr // N_SL) + (rr - N_SL * (rr // N_SL))
                     - BIAS).astype(np.int16)

        def _wrap_idx_t(pos_flat):
            # per GM macro: idxs = [GM entries] + IDX_PAD trailing zeros
            # (>=0 so the ucode's trailing-negative drop never fires), laid
            # out [16, n/16] per 16-partition group, replicated to 128 parts.
            n = GM + IDX_PAD
            blocks = pos_flat.reshape(NG, GM)
            blocks = np.concatenate(
                [blocks, np.zeros((NG, IDX_PAD), np.int16)], axis=1)
            w = blocks.reshape(NG, n // 16, 16).transpose(0, 2, 1)  # [NG,16,n/16]
            w = np.concatenate([w] * 8, axis=1)      # [NG, 128, n/16]
            return np.ascontiguousarray(
                w.transpose(1, 0, 2).reshape(128, NG * (n // 16)))

        m = dict(
            x0c_a=x0c_a,
            x0r_a=x0e[r0["idx"]],
            x1c_a=x1e[c1["idx"]],
            x2r_a=x2e[r2["idx"]],
            xT=xT,
            relC=relC,
            rel0c=np.ascontiguousarray(c0["rel"].T).astype(BF16),
            rel0r=np.ascontiguousarray(r0["rel"].T).astype(BF16),
            rel1c=np.ascontiguousarray(c1["rel"].T).astype(BF16),
            rel2r=np.ascontiguousarray(r2["rel"].T).astype(BF16),
            rpos=_wrap_idx_t(rpos_flat),
            inv_c0=_slice_pad(inv_c0, slice(c * M_SL, (c + 1) * M_SL), MP).reshape(1, MP),
            inv_r0=_slice_pad(inv_r0, slice(c * N_SL, (c + 1) * N_SL), NP).reshape(1, NP),
            inv_c1=_slice_pad(inv_c1, slice(c * M_SL, (c + 1) * M_SL), MP).reshape(1, MP),
            inv_r2=_slice_pad(inv_r2, slice(c * N_SL, (c + 1) * N_SL), NP).reshape(1, NP),
            iota_b=iota_b,
            iota_p=iota_p,
            ident_f=ident_f,
            ident_b=ident_b,
            ones_f=ones_f,
            ones_row=ones_row,
            th10=th["theta_10"], th1x0_10=th["theta_1x0_10"],
            th01=th["theta_01"], th2x0_01=th["theta_2x0_01"],
            th11=th["theta_11"], th1x0_11=th["theta_1x0_11"],
            th2x0_11=th["theta_2x0_11"],
            th00b=th00b,
            thbT=thbT,
        )
        # per-entry output position: entry p lives in slot (w_p, t_p - t0(w_p))
        # of ITS OWN window (ct one-hot only valid there), lane p%128.
        clen = c0["clen"]
        if clen:
            p_arr = np.arange(clen)
            w_arr = c0["loc"] // WIN
            t0_arr = np.array([t0 for (t0, _) in c0["wt"]], np.int64)[w_arr]
            slot_arr = w_arr * st0c["kmax"] + (p_arr // 128 - t0_arr)
            epos = slot_arr * 128 + p_arr % 128
        else:
            epos = np.zeros(0, np.int64)
        in_maps.append(m)
        post.append(dict(epos=epos, clen=clen, corder=c0["corder"]))

    meta = dict(
        S0c=S0c, S0r=S0r, S1c=S1c, S2r=S2r,
        K0c=st0c["kmax"], K0r=st0r["kmax"], K1c=st1c["kmax"], K2r=st2r["kmax"],
        NWc=NWc, NWr=NWr, MP=MP, NP=NP,
        NNZ0=NNZ0, NNZ1=NNZ1, NNZ2=NNZ2,
    )
    return meta, in_maps, post


# --------------------------------------------------------------------------
# device program
# --------------------------------------------------------------------------

_PROG_CACHE = {}


def _build_program(meta, debug=False):
    key = (tuple(sorted(meta.items())), debug)
    if key in _PROG_CACHE:
        return _PROG_CACHE[key]

    S0c, S0r, S1c, S2r = meta["S0c"], meta["S0r"], meta["S1c"], meta["S2r"]
    K0c, K0r, K1c, K2r = meta["K0c"], meta["K0r"], meta["K1c"], meta["K2r"]
    NWc, NWr = meta["NWc"], meta["NWr"]
    MP, NP = meta["MP"], meta["NP"]
    E = S0c * 128
    NG = E // GM
    RT_ROWS = NCORES * NP
    BIAS = 32767 if RT_ROWS > 32767 else 0
    NIDX = GM + IDX_PAD
    dt = mybir.dt

    nc = bacc.Bacc("TRN2", target_bir_lowering=False, debug=False,
                   num_devices=NCORES, num_swdge_queues=4)

    def din(name, shape, dty):
        return nc.dram_tensor(name, list(shape), dty, kind="ExternalInput")

    x0c_a = din("x0c_a", [S0c, 128, U], dt.bfloat16)
    x0r_a = din("x0r_a", [S0r, 128, U], dt.bfloat16)
    x1c_a = din("x1c_a", [S1c, 128, U], dt.bfloat16)
    x2r_a = din("x2r_a", [S2r, 128, U], dt.bfloat16)
    xT = din("xT", [U, E], dt.bfloat16)
    relC = din("relC", [1, E], dt.bfloat16)
    rel0c = din("rel0c", [128, S0c], dt.bfloat16)
    rel0r = din("rel0r", [128, S0r], dt.bfloat16)
    rel1c = din("rel1c", [128, S1c], dt.bfloat16)
    rel2r = din("rel2r", [128, S2r], dt.bfloat16)
    rpos = din("rpos", [128, NG * (NIDX // 16)], dt.int16)
    inv_c0 = din("inv_c0", [1, MP], dt.float32)
    inv_r0 = din("inv_r0", [1, NP], dt.float32)
    inv_c1 = din("inv_c1", [1, MP], dt.float32)
    inv_r2 = din("inv_r2", [1, NP], dt.float32)
    iota_b = din("iota_b", [128, 128], dt.bfloat16)
    iota_p = din("iota_p", [128, 1], dt.bfloat16)
    ident_f = din("ident_f", [128, 128], dt.float32)
    ident_b = din("ident_b", [128, 128], dt.bfloat16)
    ones_f = din("ones_f", [1, U], dt.float32)
    ones_row = din("ones_row", [1, 128], dt.bfloat16)
    th10 = din("th10", [U, U], dt.float32)
    th1x0_10 = din("th1x0_10", [U, U], dt.float32)
    th01 = din("th01", [U, U], dt.float32)
    th2x0_01 = din("th2x0_01", [U, U], dt.float32)
    th11 = din("th11", [U, U], dt.float32)
    th1x0_11 = din("th1x0_11", [U, U], dt.float32)
    th2x0_11 = din("th2x0_11", [U, U], dt.float32)
    th00b = din("th00b", [U, U], dt.bfloat16)
    thbT = din("thbT", [U, 1], dt.float32)

    out_d = nc.dram_tensor("out_d", [U, E], dt.float32, kind="ExternalOutput")
    if debug:
        oht_dump = nc.dram_tensor("oht_dump", [128, CH], dt.float32,
                                  kind="ExternalOutput")
        ct_dump = nc.dram_tensor("ct_dump", [128, NWc, U], dt.float32,
                                 kind="ExternalOutput")
        rtt_dump = nc.dram_tensor("rtt_dump", [RT_ROWS, 128], dt.float32,
                                  kind="ExternalOutput")

    TOT = 2 * (MP + NP)  # free-dim length of the transposed sums buffer
    off_c0, off_r0, off_c1, off_r2 = 0, MP, MP + NP, MP + NP + MP

    with tile.TileContext(nc) as tc:
        import contextlib
        with contextlib.ExitStack() as ctx:
            pp = ctx.enter_context(tc.tile_pool(name="persist", bufs=1))
            dram = ctx.enter_context(tc.tile_pool(name="dram", bufs=1, space="DRAM"))

            # SBUF freed after phase B (sums + inv rows are big)
            pab_cm = tc.tile_pool(name="pab", bufs=1)
            pab = pab_cm.__enter__()
            sumT = pab.tile([U, TOT], dt.float32)
            iota_t = pp.tile([128, 128], dt.bfloat16)
            nc.sync.dma_start(out=iota_t[:], in_=iota_b.ap())
            iotaP_t = pp.tile([128, 1], dt.bfloat16)
            nc.sync.dma_start(out=iotaP_t[:], in_=iota_p.ap())
            ident_t = pp.tile([128, 128], dt.float32)
            nc.sync.dma_start(out=ident_t[:], in_=ident_f.ap())
            identb_t = pp.tile([128, 128], dt.bfloat16)
            nc.sync.dma_start(out=identb_t[:], in_=ident_b.ap())
            ones_t = pp.tile([1, U], dt.float32)
            nc.sync.dma_start(out=ones_t[:], in_=ones_f.ap())
            onesr_t = pp.tile([1, 128], dt.bfloat16)
            nc.sync.dma_start(out=onesr_t[:], in_=ones_row.ap())
            ths = {}
            for nm, t in (("th10", th10), ("th1x0_10", th1x0_10), ("th01", th01),
                          ("th2x0_01", th2x0_01), ("th11", th11),
                          ("th1x0_11", th1x0_11), ("th2x0_11", th2x0_11)):
                ths[nm] = pp.tile([U, U], dt.float32, name=nm + "_t")
                nc.sync.dma_start(out=ths[nm][:], in_=t.ap())
            th00_t = pp.tile([U, U], dt.bfloat16)
            nc.sync.dma_start(out=th00_t[:], in_=th00b.ap())
            thb_t = pp.tile([U, 1], dt.float32)
            nc.sync.dma_start(out=thb_t[:], in_=thbT.ap())

            # ---------------- phases A+B interleaved ------------------------
            # Row/t1 streams first -> rt table + collectives early, so the
            # Pool engine can start phase-C gathers while the t0-col stream
            # and ct table still run on PE/DVE.
            ct_stage = pp.tile([128, NWc, U], dt.float32)
            ct_stageb = pp.tile([128, NWc, U], dt.bfloat16)

            rt_bf_slice = dram.tile([NP, 128], dt.bfloat16)
            rt_bf_all = dram.tile([RT_ROWS, 128], dt.bfloat16,
                                  addr_space="Shared")

            with tc.tile_pool(name="pa", bufs=3) as pa, \
                 tc.tile_pool(name="poh", bufs=2) as poh, \
                 tc.tile_pool(name="pas", bufs=2, space="PSUM") as pas, \
                 tc.tile_pool(name="prel", bufs=1) as prel, \
                 tc.tile_pool(name="pb", bufs=2) as pb, \
                 tc.tile_pool(name="pbs", bufs=1, space="PSUM") as pbs:

                KC = 32  # slots per oh/x chunk

                def run_stream(si, xa, rel_d, K, NW, soff, S):
                    rel_t = prel.tile([128, S], dt.bfloat16, name=f"rel_t{si}",
                                      tag=f"rel{si}")
                    nc.sync.dma_start(out=rel_t[:], in_=rel_d.ap())
                    for w in range(NW):
                        pw = pas.tile([U, 128], dt.float32, space="PSUM", tag="pw")
                        for k0 in range(0, K, KC):
                            kc = min(KC, K - k0)
                            s0 = w * K + k0
                            xw = pa.tile([128, KC, U], dt.bfloat16, tag="xw")
                            nc.sync.dma_start(
                                out=xw[:, :kc, :],
                                in_=xa.ap()[s0:s0 + kc].rearrange("s p f -> p s f"))
                            # batched one-hot build: [128, kc, 128], one DVE op
                            oh = poh.tile([128, KC, 128], dt.bfloat16, tag="oh")
                            in0 = (rel_t[:, s0:s0 + kc]
                                   .rearrange("p (k f) -> p k f", f=1)
                                   .to_broadcast([128, kc, 128]))
                            in1 = (iota_t[:]
                                   .rearrange("p (k f) -> p k f", k=1)
                                   .to_broadcast([128, kc, 128]))
                            nc.vector.tensor_tensor(
                                out=oh[:, :kc, :], in0=in0, in1=in1,
                                op=mybir.AluOpType.is_equal)
                            for k in range(kc):
                                nc.tensor.matmul(
                                    pw[:], lhsT=xw[:, k, :], rhs=oh[:, k, :],
                                    start=(k0 + k == 0),
                                    stop=(k0 + k == K - 1))
                        nc.vector.tensor_copy(
                            out=sumT[:, soff + w * 128: soff + (w + 1) * 128],
                            in_=pw[:])

                invs = {}
                for nm, t, ln in (("inv_c0", inv_c0, MP), ("inv_r0", inv_r0, NP),
                                  ("inv_c1", inv_c1, MP), ("inv_r2", inv_r2, NP)):
                    invs[nm] = pab.tile([1, ln], dt.float32, name=nm + "_t")
                    nc.sync.dma_start(out=invs[nm][:], in_=t.ap())

                rt_stage = pab.tile([128, NWr, 128], dt.bfloat16)
                nc.vector.memset(rt_stage[:], 0.0)

                # --- early streams: everything the rt table + totals need ---
                run_stream(1, x0r_a, rel0r, K0r, NWr, off_r0, S0r)
                run_stream(2, x1c_a, rel1c, K1c, NWc, off_c1, S1c)
                run_stream(3, x2r_a, rel2r, K2r, NWr, off_r2, S2r)

                def table_chunk(ci, inv_a, inv_b, soff_a, soff_b, thA, thB,
                                add_g, is_ct):
                    sl = slice(ci * 128, (ci + 1) * 128)
                    pr = pbs.tile([U, 128], dt.float32, space="PSUM", tag="pr")
                    nc.tensor.matmul(pr[:], lhsT=ones_t[:], rhs=inv_a[:, sl],
                                     start=True, stop=True)
                    m0 = pb.tile([U, 128], dt.float32, tag="m0")
                    nc.vector.tensor_mul(out=m0[:],
                                         in0=sumT[:, soff_a + ci * 128:
                                                  soff_a + (ci + 1) * 128],
                                         in1=pr[:])
                    pr2 = pbs.tile([U, 128], dt.float32, space="PSUM", tag="pr2")
                    nc.tensor.matmul(pr2[:], lhsT=ones_t[:], rhs=inv_b[:, sl],
                                     start=True, stop=True)
                    m1 = pb.tile([U, 128], dt.float32, tag="m1")
                    nc.vector.tensor_mul(out=m1[:],
                                         in0=sumT[:, soff_b + ci * 128:
                                                  soff_b + (ci + 1) * 128],
                                         in1=pr2[:])
                    pc = pbs.tile([U, 128], dt.float32, space="PSUM", tag="pc")
                    nc.tensor.matmul(pc[:], lhsT=thA[:], rhs=m0[:],
                                     start=True, stop=False)
                    nc.tensor.matmul(pc[:], lhsT=thB[:], rhs=m1[:],
                                     start=False, stop=True)
                    cf = pb.tile([U, 128], dt.float32, tag="cf")
                    if add_g:
                        nc.vector.tensor_tensor(
                            out=cf[:], in0=pc[:],
                            in1=g_t[:].to_broadcast([U, 128]),
                            op=mybir.AluOpType.add)
                    else:
                        nc.vector.tensor_copy(out=cf[:], in_=pc[:])
                    pt = pbs.tile([128, U], dt.float32, space="PSUM", tag="pt")
                    nc.tensor.transpose(out=pt[:], in_=cf[:],
                                        identity=ident_t[:U, :U])
                    if is_ct:
                        nc.vector.tensor_copy(out=ct_stage[:, ci, :], in_=pt[:])
                        nc.vector.tensor_copy(out=ct_stageb[:, ci, :], in_=pt[:])
                    else:
                        nc.vector.tensor_copy(out=rt_stage[:, ci, :U], in_=pt[:])

                # rt table first, then AllGather -> unblocks phase-C gathers
                for ci in range(NWr):
                    table_chunk(ci, invs["inv_r0"], invs["inv_r2"], off_r0,
                                off_r2, ths["th01"], ths["th2x0_01"], False,
                                False)
                nc.sync.dma_start(
                    out=rt_bf_slice[0:NP].rearrange("(c p) f -> p c f", p=128),
                    in_=rt_stage[:])
                nc.gpsimd.collective_compute(
                    "AllGather", mybir.AluOpType.bypass,
                    ins=[rt_bf_slice.opt()], outs=[rt_bf_all.opt()],
                    replica_groups=[list(range(NCORES))])

                # grand totals AFTER the AllGather on the pool queue, so the
                # phase-C gathers are only gated by the AllGather itself.
                # t0 total via ROW sums (same value), t1 col, t2 row.
                totL = pp.tile([U, 4], dt.float32)
                nc.vector.memset(totL[:], 0.0)
                nc.vector.tensor_reduce(
                    out=totL[:, 0:1], in_=sumT[:, off_r0:off_r0 + NP],
                    axis=mybir.AxisListType.X, op=mybir.AluOpType.add)
                nc.vector.tensor_reduce(
                    out=totL[:, 1:2], in_=sumT[:, off_c1:off_c1 + MP],
                    axis=mybir.AxisListType.X, op=mybir.AluOpType.add)
                nc.vector.tensor_reduce(
                    out=totL[:, 2:3], in_=sumT[:, off_r2:off_r2 + NP],
                    axis=mybir.AxisListType.X, op=mybir.AluOpType.add)

                totb = dram.tile([U, 4], dt.float32)
                totg = dram.tile([U, 4], dt.float32, addr_space="Shared")
                nc.gpsimd.dma_start(out=totb[:], in_=totL[:])
                nc.gpsimd.collective_compute(
                    "AllReduce", mybir.AluOpType.add,
                    ins=[totb.opt()], outs=[totg.opt()],
                    replica_groups=[list(range(NCORES))])
                totG = pp.tile([U, 4], dt.float32)
                nc.gpsimd.dma_start(out=totG[:], in_=totg[:])

                # --- late: t0 col stream + global term + ct table ----------
                run_stream(0, x0c_a, rel0c, K0c, NWc, off_c0, S0c)

                mv = pp.tile([U, 4], dt.float32)
                for j, nnz in ((0, meta["NNZ0"]), (1, meta["NNZ1"]),
                               (2, meta["NNZ2"])):
                    nc.vector.tensor_scalar_mul(
                        out=mv[:, j:j + 1], in0=totG[:, j:j + 1],
                        scalar1=float(1.0 / nnz))
                gp = pbs.tile([U, 1], dt.float32, space="PSUM", tag="gp")
                nc.tensor.matmul(gp[:], lhsT=ths["th11"][:], rhs=mv[:, 0:1],
                                 start=True, stop=False)
                nc.tensor.matmul(gp[:], lhsT=ths["th1x0_11"][:], rhs=mv[:, 1:2],
                                 start=False, stop=False)
                nc.tensor.matmul(gp[:], lhsT=ths["th2x0_11"][:], rhs=mv[:, 2:3],
                                 start=False, stop=True)
                g_t = pp.tile([U, 1], dt.float32)
                nc.vector.tensor_add(out=g_t[:], in0=gp[:], in1=thb_t[:])

                for ci in range(NWc):
                    table_chunk(ci, invs["inv_c0"], invs["inv_c1"], off_c0,
                                off_c1, ths["th10"], ths["th1x0_10"], True, True)
                if debug:
                    nc.sync.dma_start(out=ct_dump.ap(), in_=ct_stage[:])
                    with tc.tile_pool(name="pdbg", bufs=2) as pdbg:
                        for b in range(RT_ROWS // 128):
                            dtile = pdbg.tile([128, 128], dt.bfloat16, tag="dt")
                            nc.sync.dma_start(
                                out=dtile[:],
                                in_=rt_bf_all[b * 128:(b + 1) * 128])
                            dtf = pdbg.tile([128, 128], dt.float32, tag="dtf")
                            nc.vector.tensor_copy(out=dtf[:], in_=dtile[:])
                            nc.sync.dma_start(
                                out=rtt_dump.ap()[b * 128:(b + 1) * 128],
                                in_=dtf[:])

            pab_cm.__exit__(None, None, None)

            # ---------------- phase C: per-entry output (transposed) --------
            with tc.tile_pool(name="pcx", bufs=3) as pcx, \
                 tc.tile_pool(name="pcg", bufs=4) as pcg, \
                 tc.tile_pool(name="pcr", bufs=2) as pcr, \
                 tc.tile_pool(name="pco", bufs=2) as pco, \
                 tc.tile_pool(name="pct", bufs=3) as pct, \
                 tc.tile_pool(name="pcs", bufs=3, space="PSUM") as pcs, \
                 tc.tile_pool(name="pcs2", bufs=2, space="PSUM") as pcs2, \
                 tc.tile_pool(name="ppos", bufs=1) as ppos:

                rpos_t = ppos.tile([128, NG * (NIDX // 16)], dt.int16)
                nc.sync.dma_start(out=rpos_t[:], in_=rpos.ap())

                gather_src = rt_bf_all[BIAS:] if BIAS else rt_bf_all[0:]

                K = K0c
                for g in range(NG):
                    # rt rows for this macro, entry-major bf16 [128, slots, 128]
                    rtg = pcg.tile([128, NIDX // 128, 128], dt.bfloat16,
                                   tag="rtg")
                    nc.gpsimd.dma_gather(
                        out_ap=rtg[:], in_ap=gather_src,
                        idxs_ap=rpos_t[:, g * (NIDX // 16):(g + 1) * (NIDX // 16)],
                        num_idxs=NIDX, num_idxs_reg=NIDX, elem_size=128,
                        single_packet=False,
                        queue_num=g % 4)
                    # x stream chunk [64, GM]
                    xc = pcx.tile([U, GM], dt.bfloat16, tag="xc")
                    nc.sync.dma_start(out=xc[:], in_=xT.ap()[:, g * GM:(g + 1) * GM])
                    # rel chunk [1, GM]
                    rc = pcr.tile([1, GM], dt.bfloat16, tag="rc")
                    nc.sync.dma_start(out=rc[:], in_=relC.ap()[:, g * GM:(g + 1) * GM])
                    ot = pco.tile([U, GM], dt.float32, tag="ot")
                    for j in range(GM // CH):
                        e0 = g * GM + j * CH
                        # replicate rel across partitions: K=1 ones matmul
                        rp = pcs2.tile([128, CH], dt.float32, space="PSUM",
                                       tag="rp")
                        nc.tensor.matmul(rp[:], lhsT=onesr_t[:],
                                         rhs=rc[:, j * CH:(j + 1) * CH],
                                         start=True, stop=True)
                        ohT = pct.tile([128, CH], dt.bfloat16, tag="ohT")
                        nc.vector.tensor_tensor(
                            out=ohT[:], in0=rp[:],
                            in1=iotaP_t[:].to_broadcast([128, CH]),
                            op=mybir.AluOpType.is_equal)
                        if debug and g == 0 and j == 0:
                            ohTd = pct.tile([128, CH], dt.float32, tag="ohTd")
                            nc.vector.tensor_copy(out=ohTd[:], in_=ohT[:])
                            nc.sync.dma_start(out=oht_dump.ap(), in_=ohTd[:])
                        # z + ct + rt accumulation
                        acc = pcs.tile([U, CH], dt.float32, space="PSUM",
                                       tag="acc")
                        nc.tensor.matmul(acc[:], lhsT=th00_t[:],
                                         rhs=xc[:, j * CH:(j + 1) * CH],
                                         start=True, stop=False)
                        for q in range(CH // 128):
                            s = (e0 + q * 128) // 128  # global slot
                            w = min(s // K, NWc - 1)   # pad slots: ohT is 0
                            nc.tensor.matmul(
                                acc[:, q * 128:(q + 1) * 128],
                                lhsT=ct_stageb[:, w, :],
                                rhs=ohT[:, q * 128:(q + 1) * 128],
                                start=False, stop=False)
                            # rt rows: transpose gathered slot via identity
                            nc.tensor.matmul(
                                acc[:, q * 128:(q + 1) * 128],
                                lhsT=rtg[:, (j * CH) // 128 + q, 0:U],
                                rhs=identb_t[:],
                                start=False, stop=True)
                        nc.scalar.activation(
                            out=ot[:, j * CH:(j + 1) * CH], in_=acc[:],
                            func=mybir.ActivationFunctionType.Relu)
                    nc.sync.dma_start(
                        out=out_d.ap()[:, g * GM:(g + 1) * GM], in_=ot[:])

    nc.compile()
    _PROG_CACHE[key] = nc
    return nc


# --------------------------------------------------------------------------
# entry point
# --------------------------------------------------------------------------

def _run(inputs, dims, trace=False, debug=False):
    meta, in_maps, post = _prepare(inputs, dims)
    nc = _build_program(meta, debug=debug)
    res = run_bass_kernel_spmd(nc, in_maps, core_ids=list(range(NCORES)),
                               trace=trace)
    NNZ0 = dims["NNZ0"]
    out = np.empty((NNZ0, U), np.float32)
    for c in range(NCORES):
        o = res.results[c]["out_d"]            # [64, E]
        p = post[c]
        if p["clen"] == 0:
            continue
        out[p["corder"]] = o[:, p["epos"]].T
    return out, res


def kernel(**inputs):
    out, _ = _run(inputs, FULL_DIMS, trace=False)
    return out


# ------- helpers for test harness ------------------------------------------

def install_ntff_hook():
    """Enable NTFF profiling under axon (exec_time_ns in results)."""
    try:
        import antenv
        mod = types.ModuleType("antenv.axon_hooks")
        _h = [None]
        mod.set_axon_ntff_profile_hook = lambda h: _h.__setitem__(0, h)
        mod.get_axon_ntff_profile_hook = lambda: _h[0]
        sys.modules["antenv.axon_hooks"] = mod
        antenv.axon_hooks = mod
        from trn_agent_boot.trn_boot import _ntff_profile_via_ctypes
        mod.set_axon_ntff_profile_hook(
            _ntff_profile_via_ctypes("/opt/axon/libaxon_pjrt.so"))
        return True
    except Exception as e:  # pragma: no cover
        print("ntff hook install failed:", e)
        return False


def ref_numpy(inputs, dims):
    """Numpy port of the reference (for arbitrary dims)."""
    N, M = dims["N"], dims["M"]
    x0 = np.asarray(inputs["t0_values"], np.float64)
    x1 = np.asarray(inputs["t1_values"], np.float64)
    x2 = np.asarray(inputs["t2_values"], np.float64)
    tr = np.asarray(inputs["t0_rows"]); tcl = np.asarray(inputs["t0_cols"])
    t1c = np.asarray(inputs["t1_cols"]); t2r = np.asarray(inputs["t2_rows"])

    def segmean(v, ids, n):
        s = np.zeros((n, v.shape[1])); np.add.at(s, ids, v)
        c = np.bincount(ids, minlength=n).astype(np.float64)
        return s / (c + EPS)[:, None]

    th = {k: np.asarray(inputs[k], np.float64) for k in
          ("theta_00", "theta_10", "theta_01", "theta_11", "theta_1x0_10",
           "theta_1x0_11", "theta_2x0_01", "theta_2x0_11")}
    vals = x0 @ th["theta_00"]
    vals += (segmean(x0, tcl, M) @ th["theta_10"])[tcl]
    vals += (segmean(x0, tr, N) @ th["theta_01"])[tr]
    vals += x0.mean(0) @ th["theta_11"]
    vals += (segmean(x1, t1c, M) @ th["theta_1x0_10"])[tcl]
    vals += x1.mean(0) @ th["theta_1x0_11"]
    vals += (segmean(x2, t2r, N) @ th["theta_2x0_01"])[tr]
    vals += x2.mean(0) @ th["theta_2x0_11"]
    vals += np.asarray(inputs["theta_b"], np.float64)
    return np.maximum(vals, 0.0).astype(np.float32)


# revision 27
# speedup vs baseline: 1.1327x; 1.1327x over previous
"""Trainium2 Bass kernel for nn_ExchangeableLayer (segment_reduce).

out[e] = relu( x[e] @ th00
             + (segmean(t0, cols) @ th10)[c_e]
             + (segmean(t0, rows) @ th01)[r_e]
             + (segmean(t1, t1cols) @ th1x0_10)[c_e]
             + (segmean(t2, t2rows) @ th2x0_01)[r_e]
             + mean(t0) @ th11 + mean(t1) @ th1x0_11 + mean(t2) @ th2x0_11
             + theta_b )

Strategy: sort entries by segment id on host, shard contiguously by segment
range across 8 cores.  Per core:
  A) segment sums via PE one-hot matmuls into per-128-segment PSUM windows
     (tables kept transposed [64, segs] in SBUF); one-hots built in ONE
     batched DVE op per window.
  B) scale by host-precomputed 1/(cnt+eps), apply thetas (PE), fold the
     global-mean term into the col table; col table stays resident in SBUF
     [128seg, NWc, 64]; row table cast to bf16 rows padded to 128 elems and
     AllGathered to a full [8*NP, 128] bf16 DRAM table.
  C) per-entry phase, TRANSPOSED layout [64, entries], col-sorted order:
       z   = th00^T @ xT            (one stationary load, streams entries)
       ct  = ct_win^T @ ohT         (accumulated into same PSUM)
       ohT built via K=1 ones-matmul replication of rel + DVE is_equal
       rt  = dma_gather(transpose=True) of bf16 row-table rows; rr-queued
             across 4 SWDGE queues
       out = relu(psum + rt) -> [64, E] f32 -> DRAM; host un-transposes.
"""

import math
import os
import sys
import types

import numpy as np

for _p in ("/root/.axon_site/_ro/trn_rl_repo", "/opt/trn_rl_repo"):
    if os.path.isdir(_p) and _p not in sys.path:
        sys.path.append(_p)

import ml_dtypes

import concourse.bass as bass
import concourse.mybir as mybir
from concourse import bacc, tile
from concourse.bass_utils import run_bass_kernel_spmd

BF16 = ml_dtypes.bfloat16
F32 = np.float32
NCORES = 8
U = 64
WIN = 128
EPS = 1e-10
GM = 4096          # phase-C gather/stream macro (entries)
CH = 512           # phase-C psum chunk (entries)
IDX_PAD = 128      # trailing non-negative idxs per gather call

# Full-size problem dims (the graded problem).
FULL_DIMS = dict(N=50000, M=10000, NNZ0=1_000_000, NNZ1=500_000, NNZ2=500_000)


# --------------------------------------------------------------------------
# host-side preparation
# --------------------------------------------------------------------------

def _prep_stream(ids, seg_sl):
    """Sort entries by id, shard contiguously at multiples of seg_sl."""
    order = np.argsort(ids, kind="stable").astype(np.int64)
    sids = ids[order]
    bounds = np.searchsorted(sids, seg_sl * np.arange(NCORES + 1)).astype(np.int64)
    NW = -(-seg_sl // WIN)
    cores = []
    kmax = 1
    for c in range(NCORES):
        lo, hi = int(bounds[c]), int(bounds[c + 1])
        clen = hi - lo
        loc = (sids[lo:hi] - seg_sl * c).astype(np.int64)
        tc = -(-clen // 128)
        ws = np.searchsorted(loc, WIN * np.arange(NW + 1))
        wt = []
        for w in range(NW):
            a, b = int(ws[w]), int(ws[w + 1])
            if b > a:
                t0, t1 = a // 128, (b - 1) // 128
                wt.append((t0, t1 - t0 + 1))
                kmax = max(kmax, t1 - t0 + 1)
            else:
                wt.append((0, 0))
        cores.append(dict(clen=clen, loc=loc, corder=order[lo:hi], tc=tc, wt=wt))
    return dict(NW=NW, kmax=kmax, cores=cores)


def _mat_stream(stream, S, nnz):
    """Materialize per-core slot arrays: entry indices + rel ids."""
    NW, K = stream["NW"], stream["kmax"]
    for core in stream["cores"]:
        idx = np.full((S, 128), nnz, np.int64)
        rel = np.full((S, 128), -1.0, np.float32)
        tc, clen = core["tc"], core["clen"]
        locp = np.full(tc * 128, -(10 ** 6), np.int64)
        locp[:clen] = core["loc"]
        cordp = np.full(tc * 128, nnz, np.int64)
        cordp[:clen] = core["corder"]
        first_slot = np.full(max(tc, 1), -1, np.int64)
        for w, (t0, nt) in enumerate(core["wt"]):
            for k in range(nt):
                t = t0 + k
                s = w * K + k
                idx[s] = cordp[t * 128:(t + 1) * 128]
                rel[s] = locp[t * 128:(t + 1) * 128] - WIN * w
                if first_slot[t] < 0:
                    first_slot[t] = s
        core["idx"] = idx
        core["rel"] = rel
        core["first_slot"] = first_slot


def _prepare(inputs, dims):
    """All host-side metadata + per-core input arrays."""
    N, M = dims["N"], dims["M"]
    NNZ0, NNZ1, NNZ2 = dims["NNZ0"], dims["NNZ1"], dims["NNZ2"]
    M_SL, N_SL = M // NCORES, N // NCORES

    t0_rows = np.asarray(inputs["t0_rows"], np.int64)
    t0_cols = np.asarray(inputs["t0_cols"], np.int64)
    t1_cols = np.asarray(inputs["t1_cols"], np.int64)
    t2_rows = np.asarray(inputs["t2_rows"], np.int64)

    st0c = _prep_stream(t0_cols, M_SL)
    st0r = _prep_stream(t0_rows, N_SL)
    st1c = _prep_stream(t1_cols, M_SL)
    st2r = _prep_stream(t2_rows, N_SL)

    # uniform slot counts; pad S0c to a multiple of GM/128 for phase C macros
    SLOTS_PER_GM = GM // 128
    S0c = -(-(st0c["NW"] * st0c["kmax"]) // SLOTS_PER_GM) * SLOTS_PER_GM
    S0r = st0r["NW"] * st0r["kmax"]
    S1c = st1c["NW"] * st1c["kmax"]
    S2r = st2r["NW"] * st2r["kmax"]

    _mat_stream(st0c, S0c, NNZ0)
    _mat_stream(st0r, S0r, NNZ0)
    _mat_stream(st1c, S1c, NNZ1)
    _mat_stream(st2r, S2r, NNZ2)

    NWc, NWr = st0c["NW"], st0r["NW"]
    MP, NP = NWc * 128, NWr * 128          # padded per-core table slice rows
    E = S0c * 128                           # phase-C padded entries per core
    NG = E // GM                            # gather macros

    x0 = np.asarray(inputs["t0_values"], np.float32)
    x1 = np.asarray(inputs["t1_values"], np.float32)
    x2 = np.asarray(inputs["t2_values"], np.float32)
    x0e = np.concatenate([x0, np.zeros((1, U), np.float32)]).astype(BF16)
    x1e = np.concatenate([x1, np.zeros((1, U), np.float32)]).astype(BF16)
    x2e = np.concatenate([x2, np.zeros((1, U), np.float32)]).astype(BF16)

    def _inv(ids, nseg):
        cnt = np.bincount(ids, minlength=nseg).astype(np.float32)
        return (1.0 / (cnt + np.float32(EPS))).astype(np.float32)

    inv_c0 = _inv(t0_cols, M)
    inv_r0 = _inv(t0_rows, N)
    inv_c1 = _inv(t1_cols, M)
    inv_r2 = _inv(t2_rows, N)

    def _slice_pad(arr, sl, pad_to):
        out = np.ones(pad_to, np.float32)
        out[: sl.stop - sl.start] = arr[sl]
        return out

    rext = np.concatenate([t0_rows, [0]])

    # shared constants
    iota_b = np.broadcast_to(np.arange(128, dtype=np.float32), (128, 128)).astype(BF16)
    iota_p = np.arange(128, dtype=np.float32).reshape(128, 1).astype(BF16)
    ident_f = np.eye(128, dtype=np.float32)
    ident_b = np.eye(128, dtype=np.float32).astype(BF16)
    ones_f = np.ones((1, U), np.float32)
    ones_row = np.ones((1, 128), np.float32).astype(BF16)
    th = {k: np.asarray(inputs[k], np.float32) for k in
          ("theta_00", "theta_10", "theta_01", "theta_11", "theta_1x0_10",
           "theta_1x0_11", "theta_2x0_01", "theta_2x0_11")}
    th00b = th["theta_00"].astype(BF16)      # [64, 64] lhsT for z
    thbT = np.asarray(inputs["theta_b"], np.float32).reshape(U, 1)

    # rt table geometry: full table rows = NCORES * NP (row-padded bf16[128])
    RT_ROWS = NCORES * NP
    BIAS = 32767 if RT_ROWS > 32767 else 0

    in_maps = []
    post = []
    for c in range(NCORES):
        c0, r0, c1, r2 = (st0c["cores"][c], st0r["cores"][c],
                          st1c["cores"][c], st2r["cores"][c])
        # phase-C x stream, transposed: [64, E] bf16 (col-sorted slot order)
        x0c_a = x0e[c0["idx"]]                      # [S0c, 128, 64] bf16
        xT = np.ascontiguousarray(
            x0c_a.transpose(2, 0, 1).reshape(U, E))

        # phase-C rel stream [1, E] bf16 (col rel within window; pad = -1)
        relC = np.ascontiguousarray(c0["rel"].reshape(1, E)).astype(BF16)

        # phase-C rt gather indices: global row id -> padded table position
        rr = rext[c0["idx"]].reshape(E)             # [E]
        rpos_flat = (NP * (rr // N_SL) + (rr - N_SL * (rr // N_SL))
                     - BIAS).astype(np.int16)

        def _wrap_idx_t(pos_flat):
            # per GM macro: idxs = [GM entries] + IDX_PAD trailing zeros
            # (>=0 so the ucode's trailing-negative drop never fires), laid
            # out [16, n/16] per 16-partition group, replicated to 128 parts.
            n = GM + IDX_PAD
            blocks = pos_flat.reshape(NG, GM)
            blocks = np.concatenate(
                [blocks, np.zeros((NG, IDX_PAD), np.int16)], axis=1)
            w = blocks.reshape(NG, n // 16, 16).transpose(0, 2, 1)  # [NG,16,n/16]
            w = np.concatenate([w] * 8, axis=1)      # [NG, 128, n/16]
            return np.ascontiguousarray(
                w.transpose(1, 0, 2).reshape(128, NG * (n // 16)))

        m = dict(
            x0c_a=x0c_a,
            x0r_a=x0e[r0["idx"]],
            x1c_a=x1e[c1["idx"]],
            x2r_a=x2e[r2["idx"]],
            xT=xT,
            relC=relC,
            rel0c=np.ascontiguousarray(c0["rel"].T).astype(BF16),
            rel0r=np.ascontiguousarray(r0["rel"].T).astype(BF16),
            rel1c=np.ascontiguousarray(c1["rel"].T).astype(BF16),
            rel2r=np.ascontiguousarray(r2["rel"].T).astype(BF16),
            rpos=_wrap_idx_t(rpos_flat),
            inv_c0=_slice_pad(inv_c0, slice(c * M_SL, (c + 1) * M_SL), MP).reshape(1, MP),
            inv_r0=_slice_pad(inv_r0, slice(c * N_SL, (c + 1) * N_SL), NP).reshape(1, NP),
            inv_c1=_slice_pad(inv_c1, slice(c * M_SL, (c + 1) * M_SL), MP).reshape(1, MP),
            inv_r2=_slice_pad(inv_r2, slice(c * N_SL, (c + 1) * N_SL), NP).reshape(1, NP),
            iota_b=iota_b,
            iota_p=iota_p,
            ident_f=ident_f,
            ident_b=ident_b,
            ones_f=ones_f,
            ones_row=ones_row,
            th10=th["theta_10"], th1x0_10=th["theta_1x0_10"],
            th01=th["theta_01"], th2x0_01=th["theta_2x0_01"],
            th11=th["theta_11"], th1x0_11=th["theta_1x0_11"],
            th2x0_11=th["theta_2x0_11"],
            th00b=th00b,
            thbT=thbT,
        )
        # per-entry output position: entry p lives in slot (w_p, t_p - t0(w_p))
        # of ITS OWN window (ct one-hot only valid there), lane p%128.
        clen = c0["clen"]
        if clen:
            p_arr = np.arange(clen)
            w_arr = c0["loc"] // WIN
            t0_arr = np.array([t0 for (t0, _) in c0["wt"]], np.int64)[w_arr]
            slot_arr = w_arr * st0c["kmax"] + (p_arr // 128 - t0_arr)
            epos = slot_arr * 128 + p_arr % 128
        else:
            epos = np.zeros(0, np.int64)
        in_maps.append(m)
        post.append(dict(epos=epos, clen=clen, corder=c0["corder"]))

    meta = dict(
        S0c=S0c, S0r=S0r, S1c=S1c, S2r=S2r,
        K0c=st0c["kmax"], K0r=st0r["kmax"], K1c=st1c["kmax"], K2r=st2r["kmax"],
        NWc=NWc, NWr=NWr, MP=MP, NP=NP,
        NNZ0=NNZ0, NNZ1=NNZ1, NNZ2=NNZ2,
    )
    return meta, in_maps, post


# --------------------------------------------------------------------------
# device program
# --------------------------------------------------------------------------

_PROG_CACHE = {}


def _build_program(meta, debug=False):
    key = (tuple(sorted(meta.items())), debug)
    if key in _PROG_CACHE:
        return _PROG_CACHE[key]

    S0c, S0r, S1c, S2r = meta["S0c"], meta["S0r"], meta["S1c"], meta["S2r"]
    K0c, K0r, K1c, K2r = meta["K0c"], meta["K0r"], meta["K1c"], meta["K2r"]
    NWc, NWr = meta["NWc"], meta["NWr"]
    MP, NP = meta["MP"], meta["NP"]
    E = S0c * 128
    NG = E // GM
    RT_ROWS = NCORES * NP
    BIAS = 32767 if RT_ROWS > 32767 else 0
    NIDX = GM + IDX_PAD
    dt = mybir.dt

    nc = bacc.Bacc("TRN2", target_bir_lowering=False, debug=False,
                   num_devices=NCORES, num_swdge_queues=4)

    def din(name, shape, dty):
        return nc.dram_tensor(name, list(shape), dty, kind="ExternalInput")

    x0c_a = din("x0c_a", [S0c, 128, U], dt.bfloat16)
    x0r_a = din("x0r_a", [S0r, 128, U], dt.bfloat16)
    x1c_a = din("x1c_a", [S1c, 128, U], dt.bfloat16)
    x2r_a = din("x2r_a", [S2r, 128, U], dt.bfloat16)
    xT = din("xT", [U, E], dt.bfloat16)
    relC = din("relC", [1, E], dt.bfloat16)
    rel0c = din("rel0c", [128, S0c], dt.bfloat16)
    rel0r = din("rel0r", [128, S0r], dt.bfloat16)
    rel1c = din("rel1c", [128, S1c], dt.bfloat16)
    rel2r = din("rel2r", [128, S2r], dt.bfloat16)
    rpos = din("rpos", [128, NG * (NIDX // 16)], dt.int16)
    inv_c0 = din("inv_c0", [1, MP], dt.float32)
    inv_r0 = din("inv_r0", [1, NP], dt.float32)
    inv_c1 = din("inv_c1", [1, MP], dt.float32)
    inv_r2 = din("inv_r2", [1, NP], dt.float32)
    iota_b = din("iota_b", [128, 128], dt.bfloat16)
    iota_p = din("iota_p", [128, 1], dt.bfloat16)
    ident_f = din("ident_f", [128, 128], dt.float32)
    ident_b = din("ident_b", [128, 128], dt.bfloat16)
    ones_f = din("ones_f", [1, U], dt.float32)
    ones_row = din("ones_row", [1, 128], dt.bfloat16)
    th10 = din("th10", [U, U], dt.float32)
    th1x0_10 = din("th1x0_10", [U, U], dt.float32)
    th01 = din("th01", [U, U], dt.float32)
    th2x0_01 = din("th2x0_01", [U, U], dt.float32)
    th11 = din("th11", [U, U], dt.float32)
    th1x0_11 = din("th1x0_11", [U, U], dt.float32)
    th2x0_11 = din("th2x0_11", [U, U], dt.float32)
    th00b = din("th00b", [U, U], dt.bfloat16)
    thbT = din("thbT", [U, 1], dt.float32)

    out_d = nc.dram_tensor("out_d", [U, E], dt.float32, kind="ExternalOutput")
    if debug:
        oht_dump = nc.dram_tensor("oht_dump", [128, CH], dt.float32,
                                  kind="ExternalOutput")
        ct_dump = nc.dram_tensor("ct_dump", [128, NWc, U], dt.float32,
                                 kind="ExternalOutput")
        rtt_dump = nc.dram_tensor("rtt_dump", [RT_ROWS, 128], dt.float32,
                                  kind="ExternalOutput")

    TOT = 2 * (MP + NP)  # free-dim length of the transposed sums buffer
    off_c0, off_r0, off_c1, off_r2 = 0, MP, MP + NP, MP + NP + MP

    with tile.TileContext(nc) as tc:
        import contextlib
        with contextlib.ExitStack() as ctx:
            pp = ctx.enter_context(tc.tile_pool(name="persist", bufs=1))
            dram = ctx.enter_context(tc.tile_pool(name="dram", bufs=1, space="DRAM"))

            # SBUF freed after phase B (sums + inv rows are big)
            pab_cm = tc.tile_pool(name="pab", bufs=1)
            pab = pab_cm.__enter__()
            sumT = pab.tile([U, TOT], dt.float32)
            iota_t = pp.tile([128, 128], dt.bfloat16)
            nc.sync.dma_start(out=iota_t[:], in_=iota_b.ap())
            iotaP_t = pp.tile([128, 1], dt.bfloat16)
            nc.sync.dma_start(out=iotaP_t[:], in_=iota_p.ap())
            ident_t = pp.tile([128, 128], dt.float32)
            nc.sync.dma_start(out=ident_t[:], in_=ident_f.ap())
            identb_t = pp.tile([128, 128], dt.bfloat16)
            nc.sync.dma_start(out=identb_t[:], in_=ident_b.ap())
            ones_t = pp.tile([1, U], dt.float32)
            nc.sync.dma_start(out=ones_t[:], in_=ones_f.ap())
            onesr_t = pp.tile([1, 128], dt.bfloat16)
            nc.sync.dma_start(out=onesr_t[:], in_=ones_row.ap())
            ths = {}
            for nm, t in (("th10", th10), ("th1x0_10", th1x0_10), ("th01", th01),
                          ("th2x0_01", th2x0_01), ("th11", th11),
                          ("th1x0_11", th1x0_11), ("th2x0_11", th2x0_11)):
                ths[nm] = pp.tile([U, U], dt.float32, name=nm + "_t")
                nc.sync.dma_start(out=ths[nm][:], in_=t.ap())
            th00_t = pp.tile([U, U], dt.bfloat16)
            nc.sync.dma_start(out=th00_t[:], in_=th00b.ap())
            thb_t = pp.tile([U, 1], dt.float32)
            nc.sync.dma_start(out=thb_t[:], in_=thbT.ap())

            # ---------------- phases A+B interleaved ------------------------
            # Row/t1 streams first -> rt table + collectives early, so the
            # Pool engine can start phase-C gathers while the t0-col stream
            # and ct table still run on PE/DVE.
            ct_stage = pp.tile([128, NWc, U], dt.float32)
            ct_stageb = pp.tile([128, NWc, U], dt.bfloat16)

            rt_bf_slice = dram.tile([NP, 128], dt.bfloat16)
            rt_bf_all = dram.tile([RT_ROWS, 128], dt.bfloat16,
                                  addr_space="Shared")

            with tc.tile_pool(name="pa", bufs=3) as pa, \
                 tc.tile_pool(name="poh", bufs=2) as poh, \
                 tc.tile_pool(name="pas", bufs=2, space="PSUM") as pas, \
                 tc.tile_pool(name="prel", bufs=1) as prel, \
                 tc.tile_pool(name="pb", bufs=2) as pb, \
                 tc.tile_pool(name="pbs", bufs=1, space="PSUM") as pbs:

                KC = 32  # slots per oh/x chunk

                def run_stream(si, xa, rel_d, K, NW, soff, S):
                    rel_t = prel.tile([128, S], dt.bfloat16, name=f"rel_t{si}",
                                      tag=f"rel{si}")
                    nc.sync.dma_start(out=rel_t[:], in_=rel_d.ap())
                    for w in range(NW):
                        pw = pas.tile([U, 128], dt.float32, space="PSUM", tag="pw")
                        for k0 in range(0, K, KC):
                            kc = min(KC, K - k0)
                            s0 = w * K + k0
                            xw = pa.tile([128, KC, U], dt.bfloat16, tag="xw")
                            nc.sync.dma_start(
                                out=xw[:, :kc, :],
                                in_=xa.ap()[s0:s0 + kc].rearrange("s p f -> p s f"))
                            # batched one-hot build: [128, kc, 128], one DVE op
                            oh = poh.tile([128, KC, 128], dt.bfloat16, tag="oh")
                            in0 = (rel_t[:, s0:s0 + kc]
                                   .rearrange("p (k f) -> p k f", f=1)
                                   .to_broadcast([128, kc, 128]))
                            in1 = (iota_t[:]
                                   .rearrange("p (k f) -> p k f", k=1)
                                   .to_broadcast([128, kc, 128]))
                            nc.vector.tensor_tensor(
                                out=oh[:, :kc, :], in0=in0, in1=in1,
                                op=mybir.AluOpType.is_equal)
                            for k in range(kc):
                                nc.tensor.matmul(
                                    pw[:], lhsT=xw[:, k, :], rhs=oh[:, k, :],
                                    start=(k0 + k == 0),
                                    stop=(k0 + k == K - 1))
                        nc.vector.tensor_copy(
                            out=sumT[:, soff + w * 128: soff + (w + 1) * 128],
                            in_=pw[:])

                invs = {}
                for nm, t, ln in (("inv_c0", inv_c0, MP), ("inv_r0", inv_r0, NP),
                                  ("inv_c1", inv_c1, MP), ("inv_r2", inv_r2, NP)):
                    invs[nm] = pab.tile([1, ln], dt.float32, name=nm + "_t")
                    nc.sync.dma_start(out=invs[nm][:], in_=t.ap())

                rt_stage = pab.tile([128, NWr, 128], dt.bfloat16)
                nc.vector.memset(rt_stage[:], 0.0)

                # --- early streams: everything the rt table + totals need ---
                run_stream(1, x0r_a, rel0r, K0r, NWr, off_r0, S0r)
                run_stream(2, x1c_a, rel1c, K1c, NWc, off_c1, S1c)
                run_stream(3, x2r_a, rel2r, K2r, NWr, off_r2, S2r)

                def table_chunk(ci, inv_a, inv_b, soff_a, soff_b, thA, thB,
                                add_g, is_ct):
                    sl = slice(ci * 128, (ci + 1) * 128)
                    pr = pbs.tile([U, 128], dt.float32, space="PSUM", tag="pr")
                    nc.tensor.matmul(pr[:], lhsT=ones_t[:], rhs=inv_a[:, sl],
                                     start=True, stop=True)
                    m0 = pb.tile([U, 128], dt.float32, tag="m0")
                    nc.vector.tensor_mul(out=m0[:],
                                         in0=sumT[:, soff_a + ci * 128:
                                                  soff_a + (ci + 1) * 128],
                                         in1=pr[:])
                    pr2 = pbs.tile([U, 128], dt.float32, space="PSUM", tag="pr2")
                    nc.tensor.matmul(pr2[:], lhsT=ones_t[:], rhs=inv_b[:, sl],
                                     start=True, stop=True)
                    m1 = pb.tile([U, 128], dt.float32, tag="m1")
                    nc.vector.tensor_mul(out=m1[:],
                                         in0=sumT[:, soff_b + ci * 128:
                                                  soff_b + (ci + 1) * 128],
                                         in1=pr2[:])
                    pc = pbs.tile([U, 128], dt.float32, space="PSUM", tag="pc")
                    nc.tensor.matmul(pc[:], lhsT=thA[:], rhs=m0[:],
                                     start=True, stop=False)
                    nc.tensor.matmul(pc[:], lhsT=thB[:], rhs=m1[:],
                                     start=False, stop=True)
                    cf = pb.tile([U, 128], dt.float32, tag="cf")
                    if add_g:
                        nc.vector.tensor_tensor(
                            out=cf[:], in0=pc[:],
                            in1=g_t[:].to_broadcast([U, 128]),
                            op=mybir.AluOpType.add)
                    else:
                        nc.vector.tensor_copy(out=cf[:], in_=pc[:])
                    pt = pbs.tile([128, U], dt.float32, space="PSUM", tag="pt")
                    nc.tensor.transpose(out=pt[:], in_=cf[:],
                                        identity=ident_t[:U, :U])
                    if is_ct:
                        nc.vector.tensor_copy(out=ct_stage[:, ci, :], in_=pt[:])
                        nc.vector.tensor_copy(out=ct_stageb[:, ci, :], in_=pt[:])
                    else:
                        nc.vector.tensor_copy(out=rt_stage[:, ci, :U], in_=pt[:])

                # rt table first, then AllGather -> unblocks phase-C gathers
                for ci in range(NWr):
                    table_chunk(ci, invs["inv_r0"], invs["inv_r2"], off_r0,
                                off_r2, ths["th01"], ths["th2x0_01"], False,
                                False)
                nc.sync.dma_start(
                    out=rt_bf_slice[0:NP].rearrange("(c p) f -> p c f", p=128),
                    in_=rt_stage[:])
                nc.gpsimd.collective_compute(
                    "AllGather", mybir.AluOpType.bypass,
                    ins=[rt_bf_slice.opt()], outs=[rt_bf_all.opt()],
                    replica_groups=[list(range(NCORES))])

                # grand totals AFTER the AllGather on the pool queue, so the
                # phase-C gathers are only gated by the AllGather itself.
                # t0 total via ROW sums (same value), t1 col, t2 row.
                totL = pp.tile([U, 4], dt.float32)
                nc.vector.memset(totL[:], 0.0)
                nc.vector.tensor_reduce(
                    out=totL[:, 0:1], in_=sumT[:, off_r0:off_r0 + NP],
                    axis=mybir.AxisListType.X, op=mybir.AluOpType.add)
                nc.vector.tensor_reduce(
                    out=totL[:, 1:2], in_=sumT[:, off_c1:off_c1 + MP],
                    axis=mybir.AxisListType.X, op=mybir.AluOpType.add)
                nc.vector.tensor_reduce(
                    out=totL[:, 2:3], in_=sumT[:, off_r2:off_r2 + NP],
                    axis=mybir.AxisListType.X, op=mybir.AluOpType.add)

                totb = dram.tile([U, 4], dt.float32)
                totg = dram.tile([U, 4], dt.float32, addr_space="Shared")
                nc.gpsimd.dma_start(out=totb[:], in_=totL[:])
                nc.gpsimd.collective_compute(
                    "AllReduce", mybir.AluOpType.add,
                    ins=[totb.opt()], outs=[totg.opt()],
                    replica_groups=[list(range(NCORES))])
                totG = pp.tile([U, 4], dt.float32)
                nc.gpsimd.dma_start(out=totG[:], in_=totg[:])

                # --- late: t0 col stream + global term + ct table ----------
                run_stream(0, x0c_a, rel0c, K0c, NWc, off_c0, S0c)

                mv = pp.tile([U, 4], dt.float32)
                for j, nnz in ((0, meta["NNZ0"]), (1, meta["NNZ1"]),
                               (2, meta["NNZ2"])):
                    nc.vector.tensor_scalar_mul(
                        out=mv[:, j:j + 1], in0=totG[:, j:j + 1],
                        scalar1=float(1.0 / nnz))
                gp = pbs.tile([U, 1], dt.float32, space="PSUM", tag="gp")
                nc.tensor.matmul(gp[:], lhsT=ths["th11"][:], rhs=mv[:, 0:1],
                                 start=True, stop=False)
                nc.tensor.matmul(gp[:], lhsT=ths["th1x0_11"][:], rhs=mv[:, 1:2],
                                 start=False, stop=False)
                nc.tensor.matmul(gp[:], lhsT=ths["th2x0_11"][:], rhs=mv[:, 2:3],
                                 start=False, stop=True)
                g_t = pp.tile([U, 1], dt.float32)
                nc.vector.tensor_add(out=g_t[:], in0=gp[:], in1=thb_t[:])

                for ci in range(NWc):
                    table_chunk(ci, invs["inv_c0"], invs["inv_c1"], off_c0,
                                off_c1, ths["th10"], ths["th1x0_10"], True, True)
                if debug:
                    nc.sync.dma_start(out=ct_dump.ap(), in_=ct_stage[:])
                    with tc.tile_pool(name="pdbg", bufs=2) as pdbg:
                        for b in range(RT_ROWS // 128):
                            dtile = pdbg.tile([128, 128], dt.bfloat16, tag="dt")
                            nc.sync.dma_start(
                                out=dtile[:],
                                in_=rt_bf_all[b * 128:(b + 1) * 128])
                            dtf = pdbg.tile([128, 128], dt.float32, tag="dtf")
                            nc.vector.tensor_copy(out=dtf[:], in_=dtile[:])
                            nc.sync.dma_start(
                                out=rtt_dump.ap()[b * 128:(b + 1) * 128],
                                in_=dtf[:])

            pab_cm.__exit__(None, None, None)

            # ---------------- phase C: per-entry output (transposed) --------
            with tc.tile_pool(name="pcx", bufs=3) as pcx, \
                 tc.tile_pool(name="pcg", bufs=3) as pcg, \
                 tc.tile_pool(name="pcr", bufs=2) as pcr, \
                 tc.tile_pool(name="pco", bufs=2) as pco, \
                 tc.tile_pool(name="pct", bufs=3) as pct, \
                 tc.tile_pool(name="pcs", bufs=3, space="PSUM") as pcs, \
                 tc.tile_pool(name="pcs2", bufs=2, space="PSUM") as pcs2, \
                 tc.tile_pool(name="ppos", bufs=1) as ppos:

                rpos_t = ppos.tile([128, NG * (NIDX // 16)], dt.int16)
                nc.sync.dma_start(out=rpos_t[:], in_=rpos.ap())

                gather_src = rt_bf_all[BIAS:] if BIAS else rt_bf_all[0:]

                K = K0c
                for g in range(NG):
                    # rt rows for this macro, entry-major bf16 [128, slots, 128]
                    rtg = pcg.tile([128, NIDX // 128, 128], dt.bfloat16,
                                   tag="rtg")
                    nc.gpsimd.dma_gather(
                        out_ap=rtg[:], in_ap=gather_src,
                        idxs_ap=rpos_t[:, g * (NIDX // 16):(g + 1) * (NIDX // 16)],
                        num_idxs=NIDX, num_idxs_reg=NIDX, elem_size=128,
                        single_packet=False,
                        queue_num=g % 4)
                    # x stream chunk [64, GM]
                    xc = pcx.tile([U, GM], dt.bfloat16, tag="xc")
                    nc.sync.dma_start(out=xc[:], in_=xT.ap()[:, g * GM:(g + 1) * GM])
                    # rel chunk [1, GM]
                    rc = pcr.tile([1, GM], dt.bfloat16, tag="rc")
                    nc.sync.dma_start(out=rc[:], in_=relC.ap()[:, g * GM:(g + 1) * GM])
                    ot = pco.tile([U, GM], dt.float32, tag="ot")
                    for j in range(GM // CH):
                        e0 = g * GM + j * CH
                        # replicate rel across partitions: K=1 ones matmul
                        rp = pcs2.tile([128, CH], dt.float32, space="PSUM",
                                       tag="rp")
                        nc.tensor.matmul(rp[:], lhsT=onesr_t[:],
                                         rhs=rc[:, j * CH:(j + 1) * CH],
                                         start=True, stop=True)
                        ohT = pct.tile([128, CH], dt.bfloat16, tag="ohT")
                        nc.vector.tensor_tensor(
                            out=ohT[:], in0=rp[:],
                            in1=iotaP_t[:].to_broadcast([128, CH]),
                            op=mybir.AluOpType.is_equal)
                        if debug and g == 0 and j == 0:
                            ohTd = pct.tile([128, CH], dt.float32, tag="ohTd")
                            nc.vector.tensor_copy(out=ohTd[:], in_=ohT[:])
                            nc.sync.dma_start(out=oht_dump.ap(), in_=ohTd[:])
                        # z + ct + rt accumulation
                        acc = pcs.tile([U, CH], dt.float32, space="PSUM",
                                       tag="acc")
                        nc.tensor.matmul(acc[:], lhsT=th00_t[:],
                                         rhs=xc[:, j * CH:(j + 1) * CH],
                                         start=True, stop=False)
                        for q in range(CH // 128):
                            s = (e0 + q * 128) // 128  # global slot
                            w = min(s // K, NWc - 1)   # pad slots: ohT is 0
                            nc.tensor.matmul(
                                acc[:, q * 128:(q + 1) * 128],
                                lhsT=ct_stageb[:, w, :],
                                rhs=ohT[:, q * 128:(q + 1) * 128],
                                start=False, stop=False)
                            # rt rows: transpose gathered slot via identity
                            nc.tensor.matmul(
                                acc[:, q * 128:(q + 1) * 128],
                                lhsT=rtg[:, (j * CH) // 128 + q, 0:U],
                                rhs=identb_t[:],
                                start=False, stop=True)
                        nc.scalar.activation(
                            out=ot[:, j * CH:(j + 1) * CH], in_=acc[:],
                            func=mybir.ActivationFunctionType.Relu)
                    nc.sync.dma_start(
                        out=out_d.ap()[:, g * GM:(g + 1) * GM], in_=ot[:])

    nc.compile()
    _PROG_CACHE[key] = nc
    return nc


# --------------------------------------------------------------------------
# entry point
# --------------------------------------------------------------------------

def _run(inputs, dims, trace=False, debug=False):
    meta, in_maps, post = _prepare(inputs, dims)
    nc = _build_program(meta, debug=debug)
    res = run_bass_kernel_spmd(nc, in_maps, core_ids=list(range(NCORES)),
                               trace=trace)
    NNZ0 = dims["NNZ0"]
    out = np.empty((NNZ0, U), np.float32)
    for c in range(NCORES):
        o = res.results[c]["out_d"]            # [64, E]
        p = post[c]
        if p["clen"] == 0:
            continue
        out[p["corder"]] = o[:, p["epos"]].T
    return out, res


def kernel(**inputs):
    out, _ = _run(inputs, FULL_DIMS, trace=False)
    return out


# ------- helpers for test harness ------------------------------------------

def install_ntff_hook():
    """Enable NTFF profiling under axon (exec_time_ns in results)."""
    try:
        import antenv
        mod = types.ModuleType("antenv.axon_hooks")
        _h = [None]
        mod.set_axon_ntff_profile_hook = lambda h: _h.__setitem__(0, h)
        mod.get_axon_ntff_profile_hook = lambda: _h[0]
        sys.modules["antenv.axon_hooks"] = mod
        antenv.axon_hooks = mod
        from trn_agent_boot.trn_boot import _ntff_profile_via_ctypes
        mod.set_axon_ntff_profile_hook(
            _ntff_profile_via_ctypes("/opt/axon/libaxon_pjrt.so"))
        return True
    except Exception as e:  # pragma: no cover
        print("ntff hook install failed:", e)
        return False


def ref_numpy(inputs, dims):
    """Numpy port of the reference (for arbitrary dims)."""
    N, M = dims["N"], dims["M"]
    x0 = np.asarray(inputs["t0_values"], np.float64)
    x1 = np.asarray(inputs["t1_values"], np.float64)
    x2 = np.asarray(inputs["t2_values"], np.float64)
    tr = np.asarray(inputs["t0_rows"]); tcl = np.asarray(inputs["t0_cols"])
    t1c = np.asarray(inputs["t1_cols"]); t2r = np.asarray(inputs["t2_rows"])

    def segmean(v, ids, n):
        s = np.zeros((n, v.shape[1])); np.add.at(s, ids, v)
        c = np.bincount(ids, minlength=n).astype(np.float64)
        return s / (c + EPS)[:, None]

    th = {k: np.asarray(inputs[k], np.float64) for k in
          ("theta_00", "theta_10", "theta_01", "theta_11", "theta_1x0_10",
           "theta_1x0_11", "theta_2x0_01", "theta_2x0_11")}
    vals = x0 @ th["theta_00"]
    vals += (segmean(x0, tcl, M) @ th["theta_10"])[tcl]
    vals += (segmean(x0, tr, N) @ th["theta_01"])[tr]
    vals += x0.mean(0) @ th["theta_11"]
    vals += (segmean(x1, t1c, M) @ th["theta_1x0_10"])[tcl]
    vals += x1.mean(0) @ th["theta_1x0_11"]
    vals += (segmean(x2, t2r, N) @ th["theta_2x0_01"])[tr]
    vals += x2.mean(0) @ th["theta_2x0_11"]
    vals += np.asarray(inputs["theta_b"], np.float64)
    return np.maximum(vals, 0.0).astype(np.float32)


# revision 28
# speedup vs baseline: 1.1491x; 1.0145x over previous
"""Trainium2 Bass kernel for nn_ExchangeableLayer (segment_reduce).

out[e] = relu( x[e] @ th00
             + (segmean(t0, cols) @ th10)[c_e]
             + (segmean(t0, rows) @ th01)[r_e]
             + (segmean(t1, t1cols) @ th1x0_10)[c_e]
             + (segmean(t2, t2rows) @ th2x0_01)[r_e]
             + mean(t0) @ th11 + mean(t1) @ th1x0_11 + mean(t2) @ th2x0_11
             + theta_b )

Strategy: sort entries by segment id on host, shard contiguously by segment
range across 8 cores.  Per core:
  A) segment sums via PE one-hot matmuls into per-128-segment PSUM windows
     (tables kept transposed [64, segs] in SBUF); one-hots built in ONE
     batched DVE op per window.
  B) scale by host-precomputed 1/(cnt+eps), apply thetas (PE), fold the
     global-mean term into the col table; col table stays resident in SBUF
     [128seg, NWc, 64]; row table cast to bf16 rows padded to 128 elems and
     AllGathered to a full [8*NP, 128] bf16 DRAM table.
  C) per-entry phase, TRANSPOSED layout [64, entries], col-sorted order:
       z   = th00^T @ xT            (one stationary load, streams entries)
       ct  = ct_win^T @ ohT         (accumulated into same PSUM)
       ohT built via K=1 ones-matmul replication of rel + DVE is_equal
       rt  = dma_gather(transpose=True) of bf16 row-table rows; rr-queued
             across 4 SWDGE queues
       out = relu(psum + rt) -> [64, E] f32 -> DRAM; host un-transposes.
"""

import math
import os
import sys
import types

import numpy as np

for _p in ("/root/.axon_site/_ro/trn_rl_repo", "/opt/trn_rl_repo"):
    if os.path.isdir(_p) and _p not in sys.path:
        sys.path.append(_p)

import ml_dtypes

import concourse.bass as bass
import concourse.mybir as mybir
from concourse import bacc, tile
from concourse.bass_utils import run_bass_kernel_spmd

BF16 = ml_dtypes.bfloat16
F32 = np.float32
NCORES = 8
U = 64
WIN = 128
EPS = 1e-10
GM = 4096          # phase-C gather/stream macro (entries)
CH = 512           # phase-C psum chunk (entries)
IDX_PAD = 128      # trailing non-negative idxs per gather call

# Full-size problem dims (the graded problem).
FULL_DIMS = dict(N=50000, M=10000, NNZ0=1_000_000, NNZ1=500_000, NNZ2=500_000)


# --------------------------------------------------------------------------
# host-side preparation
# --------------------------------------------------------------------------

def _prep_stream(ids, seg_sl):
    """Sort entries by id, shard contiguously at multiples of seg_sl."""
    order = np.argsort(ids, kind="stable").astype(np.int64)
    sids = ids[order]
    bounds = np.searchsorted(sids, seg_sl * np.arange(NCORES + 1)).astype(np.int64)
    NW = -(-seg_sl // WIN)
    cores = []
    kmax = 1
    for c in range(NCORES):
        lo, hi = int(bounds[c]), int(bounds[c + 1])
        clen = hi - lo
        loc = (sids[lo:hi] - seg_sl * c).astype(np.int64)
        tc = -(-clen // 128)
        ws = np.searchsorted(loc, WIN * np.arange(NW + 1))
        wt = []
        for w in range(NW):
            a, b = int(ws[w]), int(ws[w + 1])
            if b > a:
                t0, t1 = a // 128, (b - 1) // 128
                wt.append((t0, t1 - t0 + 1))
                kmax = max(kmax, t1 - t0 + 1)
            else:
                wt.append((0, 0))
        cores.append(dict(clen=clen, loc=loc, corder=order[lo:hi], tc=tc, wt=wt))
    return dict(NW=NW, kmax=kmax, cores=cores)


def _mat_stream(stream, S, nnz):
    """Materialize per-core slot arrays: entry indices + rel ids."""
    NW, K = stream["NW"], stream["kmax"]
    for core in stream["cores"]:
        idx = np.full((S, 128), nnz, np.int64)
        rel = np.full((S, 128), -1.0, np.float32)
        tc, clen = core["tc"], core["clen"]
        locp = np.full(tc * 128, -(10 ** 6), np.int64)
        locp[:clen] = core["loc"]
        cordp = np.full(tc * 128, nnz, np.int64)
        cordp[:clen] = core["corder"]
        first_slot = np.full(max(tc, 1), -1, np.int64)
        for w, (t0, nt) in enumerate(core["wt"]):
            for k in range(nt):
                t = t0 + k
                s = w * K + k
                idx[s] = cordp[t * 128:(t + 1) * 128]
                rel[s] = locp[t * 128:(t + 1) * 128] - WIN * w
                if first_slot[t] < 0:
                    first_slot[t] = s
        core["idx"] = idx
        core["rel"] = rel
        core["first_slot"] = first_slot


def _prepare(inputs, dims):
    """All host-side metadata + per-core input arrays."""
    N, M = dims["N"], dims["M"]
    NNZ0, NNZ1, NNZ2 = dims["NNZ0"], dims["NNZ1"], dims["NNZ2"]
    M_SL, N_SL = M // NCORES, N // NCORES

    t0_rows = np.asarray(inputs["t0_rows"], np.int64)
    t0_cols = np.asarray(inputs["t0_cols"], np.int64)
    t1_cols = np.asarray(inputs["t1_cols"], np.int64)
    t2_rows = np.asarray(inputs["t2_rows"], np.int64)

    st0c = _prep_stream(t0_cols, M_SL)
    st0r = _prep_stream(t0_rows, N_SL)
    st1c = _prep_stream(t1_cols, M_SL)
    st2r = _prep_stream(t2_rows, N_SL)

    # uniform slot counts; pad S0c to a multiple of GM/128 for phase C macros
    SLOTS_PER_GM = GM // 128
    S0c = -(-(st0c["NW"] * st0c["kmax"]) // SLOTS_PER_GM) * SLOTS_PER_GM
    S0r = st0r["NW"] * st0r["kmax"]
    S1c = st1c["NW"] * st1c["kmax"]
    S2r = st2r["NW"] * st2r["kmax"]

    _mat_stream(st0c, S0c, NNZ0)
    _mat_stream(st0r, S0r, NNZ0)
    _mat_stream(st1c, S1c, NNZ1)
    _mat_stream(st2r, S2r, NNZ2)

    NWc, NWr = st0c["NW"], st0r["NW"]
    MP, NP = NWc * 128, NWr * 128          # padded per-core table slice rows
    E = S0c * 128                           # phase-C padded entries per core
    NG = E // GM                            # gather macros

    x0 = np.asarray(inputs["t0_values"], np.float32)
    x1 = np.asarray(inputs["t1_values"], np.float32)
    x2 = np.asarray(inputs["t2_values"], np.float32)
    x0e = np.concatenate([x0, np.zeros((1, U), np.float32)]).astype(BF16)
    x1e = np.concatenate([x1, np.zeros((1, U), np.float32)]).astype(BF16)
    x2e = np.concatenate([x2, np.zeros((1, U), np.float32)]).astype(BF16)

    def _inv(ids, nseg):
        cnt = np.bincount(ids, minlength=nseg).astype(np.float32)
        return (1.0 / (cnt + np.float32(EPS))).astype(np.float32)

    inv_c0 = _inv(t0_cols, M)
    inv_r0 = _inv(t0_rows, N)
    inv_c1 = _inv(t1_cols, M)
    inv_r2 = _inv(t2_rows, N)

    def _slice_pad(arr, sl, pad_to):
        out = np.ones(pad_to, np.float32)
        out[: sl.stop - sl.start] = arr[sl]
        return out

    rext = np.concatenate([t0_rows, [0]])

    # shared constants
    iota_b = np.broadcast_to(np.arange(128, dtype=np.float32), (128, 128)).astype(BF16)
    iota_p = np.arange(128, dtype=np.float32).reshape(128, 1).astype(BF16)
    ident_f = np.eye(128, dtype=np.float32)
    ident_b = np.eye(128, dtype=np.float32).astype(BF16)
    ones_f = np.ones((1, U), np.float32)
    ones_row = np.ones((1, 128), np.float32).astype(BF16)
    th = {k: np.asarray(inputs[k], np.float32) for k in
          ("theta_00", "theta_10", "theta_01", "theta_11", "theta_1x0_10",
           "theta_1x0_11", "theta_2x0_01", "theta_2x0_11")}
    th00b = th["theta_00"].astype(BF16)      # [64, 64] lhsT for z
    thbT = np.asarray(inputs["theta_b"], np.float32).reshape(U, 1)

    # rt table geometry: full table rows = NCORES * NP (row-padded bf16[128])
    RT_ROWS = NCORES * NP
    BIAS = 32767 if RT_ROWS > 32767 else 0

    in_maps = []
    post = []
    for c in range(NCORES):
        c0, r0, c1, r2 = (st0c["cores"][c], st0r["cores"][c],
                          st1c["cores"][c], st2r["cores"][c])
        # phase-C x stream, transposed: [64, E] bf16 (col-sorted slot order)
        x0c_a = x0e[c0["idx"]]                      # [S0c, 128, 64] bf16
        xT = np.ascontiguousarray(
            x0c_a.transpose(2, 0, 1).reshape(U, E))

        # phase-C rel stream [1, E] bf16 (col rel within window; pad = -1)
        relC = np.ascontiguousarray(c0["rel"].reshape(1, E)).astype(BF16)

        # phase-C rt gather indices: global row id -> padded table position
        rr = rext[c0["idx"]].reshape(E)             # [E]
        rpos_flat = (NP * (rr // N_SL) + (rr - N_SL * (rr // N_SL))
                     - BIAS).astype(np.int16)

        def _wrap_idx_t(pos_flat):
            # per HALF-macro: idxs = [GM/2 entries] + IDX_PAD trailing zeros
            # (>=0 so the ucode's trailing-negative drop never fires), laid
            # out [16, n/16] per 16-partition group, replicated to 128 parts.
            # Two half-gathers per macro go on different SWDGE queues so
            # their Q7 descriptor generation can overlap (different core
            # pairs).
            HM = GM // 2
            n = HM + IDX_PAD
            blocks = pos_flat.reshape(2 * NG, HM)
            blocks = np.concatenate(
                [blocks, np.zeros((2 * NG, IDX_PAD), np.int16)], axis=1)
            w = blocks.reshape(2 * NG, n // 16, 16).transpose(0, 2, 1)
            w = np.concatenate([w] * 8, axis=1)      # [2NG, 128, n/16]
            return np.ascontiguousarray(
                w.transpose(1, 0, 2).reshape(128, 2 * NG * (n // 16)))

        m = dict(
            x0c_a=x0c_a,
            x0r_a=x0e[r0["idx"]],
            x1c_a=x1e[c1["idx"]],
            x2r_a=x2e[r2["idx"]],
            xT=xT,
            relC=relC,
            rel0c=np.ascontiguousarray(c0["rel"].T).astype(BF16),
            rel0r=np.ascontiguousarray(r0["rel"].T).astype(BF16),
            rel1c=np.ascontiguousarray(c1["rel"].T).astype(BF16),
            rel2r=np.ascontiguousarray(r2["rel"].T).astype(BF16),
            rpos=_wrap_idx_t(rpos_flat),
            inv_c0=_slice_pad(inv_c0, slice(c * M_SL, (c + 1) * M_SL), MP).reshape(1, MP),
            inv_r0=_slice_pad(inv_r0, slice(c * N_SL, (c + 1) * N_SL), NP).reshape(1, NP),
            inv_c1=_slice_pad(inv_c1, slice(c * M_SL, (c + 1) * M_SL), MP).reshape(1, MP),
            inv_r2=_slice_pad(inv_r2, slice(c * N_SL, (c + 1) * N_SL), NP).reshape(1, NP),
            iota_b=iota_b,
            iota_p=iota_p,
            ident_f=ident_f,
            ident_b=ident_b,
            ones_f=ones_f,
            ones_row=ones_row,
            th10=th["theta_10"], th1x0_10=th["theta_1x0_10"],
            th01=th["theta_01"], th2x0_01=th["theta_2x0_01"],
            th11=th["theta_11"], th1x0_11=th["theta_1x0_11"],
            th2x0_11=th["theta_2x0_11"],
            th00b=th00b,
            thbT=thbT,
        )
        # per-entry output position: entry p lives in slot (w_p, t_p - t0(w_p))
        # of ITS OWN window (ct one-hot only valid there), lane p%128.
        clen = c0["clen"]
        if clen:
            p_arr = np.arange(clen)
            w_arr = c0["loc"] // WIN
            t0_arr = np.array([t0 for (t0, _) in c0["wt"]], np.int64)[w_arr]
            slot_arr = w_arr * st0c["kmax"] + (p_arr // 128 - t0_arr)
            epos = slot_arr * 128 + p_arr % 128
        else:
            epos = np.zeros(0, np.int64)
        in_maps.append(m)
        post.append(dict(epos=epos, clen=clen, corder=c0["corder"]))

    meta = dict(
        S0c=S0c, S0r=S0r, S1c=S1c, S2r=S2r,
        K0c=st0c["kmax"], K0r=st0r["kmax"], K1c=st1c["kmax"], K2r=st2r["kmax"],
        NWc=NWc, NWr=NWr, MP=MP, NP=NP,
        NNZ0=NNZ0, NNZ1=NNZ1, NNZ2=NNZ2,
    )
    return meta, in_maps, post


# --------------------------------------------------------------------------
# device program
# --------------------------------------------------------------------------

_PROG_CACHE = {}


def _build_program(meta, debug=False):
    key = (tuple(sorted(meta.items())), debug)
    if key in _PROG_CACHE:
        return _PROG_CACHE[key]

    S0c, S0r, S1c, S2r = meta["S0c"], meta["S0r"], meta["S1c"], meta["S2r"]
    K0c, K0r, K1c, K2r = meta["K0c"], meta["K0r"], meta["K1c"], meta["K2r"]
    NWc, NWr = meta["NWc"], meta["NWr"]
    MP, NP = meta["MP"], meta["NP"]
    E = S0c * 128
    NG = E // GM
    RT_ROWS = NCORES * NP
    BIAS = 32767 if RT_ROWS > 32767 else 0
    NIDX = GM // 2 + IDX_PAD
    dt = mybir.dt

    nc = bacc.Bacc("TRN2", target_bir_lowering=False, debug=False,
                   num_devices=NCORES, num_swdge_queues=4)

    def din(name, shape, dty):
        return nc.dram_tensor(name, list(shape), dty, kind="ExternalInput")

    x0c_a = din("x0c_a", [S0c, 128, U], dt.bfloat16)
    x0r_a = din("x0r_a", [S0r, 128, U], dt.bfloat16)
    x1c_a = din("x1c_a", [S1c, 128, U], dt.bfloat16)
    x2r_a = din("x2r_a", [S2r, 128, U], dt.bfloat16)
    xT = din("xT", [U, E], dt.bfloat16)
    relC = din("relC", [1, E], dt.bfloat16)
    rel0c = din("rel0c", [128, S0c], dt.bfloat16)
    rel0r = din("rel0r", [128, S0r], dt.bfloat16)
    rel1c = din("rel1c", [128, S1c], dt.bfloat16)
    rel2r = din("rel2r", [128, S2r], dt.bfloat16)
    rpos = din("rpos", [128, 2 * NG * (NIDX // 16)], dt.int16)
    inv_c0 = din("inv_c0", [1, MP], dt.float32)
    inv_r0 = din("inv_r0", [1, NP], dt.float32)
    inv_c1 = din("inv_c1", [1, MP], dt.float32)
    inv_r2 = din("inv_r2", [1, NP], dt.float32)
    iota_b = din("iota_b", [128, 128], dt.bfloat16)
    iota_p = din("iota_p", [128, 1], dt.bfloat16)
    ident_f = din("ident_f", [128, 128], dt.float32)
    ident_b = din("ident_b", [128, 128], dt.bfloat16)
    ones_f = din("ones_f", [1, U], dt.float32)
    ones_row = din("ones_row", [1, 128], dt.bfloat16)
    th10 = din("th10", [U, U], dt.float32)
    th1x0_10 = din("th1x0_10", [U, U], dt.float32)
    th01 = din("th01", [U, U], dt.float32)
    th2x0_01 = din("th2x0_01", [U, U], dt.float32)
    th11 = din("th11", [U, U], dt.float32)
    th1x0_11 = din("th1x0_11", [U, U], dt.float32)
    th2x0_11 = din("th2x0_11", [U, U], dt.float32)
    th00b = din("th00b", [U, U], dt.bfloat16)
    thbT = din("thbT", [U, 1], dt.float32)

    out_d = nc.dram_tensor("out_d", [U, E], dt.float32, kind="ExternalOutput")
    if debug:
        oht_dump = nc.dram_tensor("oht_dump", [128, CH], dt.float32,
                                  kind="ExternalOutput")
        ct_dump = nc.dram_tensor("ct_dump", [128, NWc, U], dt.float32,
                                 kind="ExternalOutput")
        rtt_dump = nc.dram_tensor("rtt_dump", [RT_ROWS, 128], dt.float32,
                                  kind="ExternalOutput")

    TOT = 2 * (MP + NP)  # free-dim length of the transposed sums buffer
    off_c0, off_r0, off_c1, off_r2 = 0, MP, MP + NP, MP + NP + MP

    with tile.TileContext(nc) as tc:
        import contextlib
        with contextlib.ExitStack() as ctx:
            pp = ctx.enter_context(tc.tile_pool(name="persist", bufs=1))
            dram = ctx.enter_context(tc.tile_pool(name="dram", bufs=1, space="DRAM"))

            # SBUF freed after phase B (sums + inv rows are big)
            pab_cm = tc.tile_pool(name="pab", bufs=1)
            pab = pab_cm.__enter__()
            sumT = pab.tile([U, TOT], dt.float32)
            iota_t = pp.tile([128, 128], dt.bfloat16)
            nc.sync.dma_start(out=iota_t[:], in_=iota_b.ap())
            iotaP_t = pp.tile([128, 1], dt.bfloat16)
            nc.sync.dma_start(out=iotaP_t[:], in_=iota_p.ap())
            ident_t = pp.tile([128, 128], dt.float32)
            nc.sync.dma_start(out=ident_t[:], in_=ident_f.ap())
            identb_t = pp.tile([128, 128], dt.bfloat16)
            nc.sync.dma_start(out=identb_t[:], in_=ident_b.ap())
            ones_t = pp.tile([1, U], dt.float32)
            nc.sync.dma_start(out=ones_t[:], in_=ones_f.ap())
            onesr_t = pp.tile([1, 128], dt.bfloat16)
            nc.sync.dma_start(out=onesr_t[:], in_=ones_row.ap())
            ths = {}
            for nm, t in (("th10", th10), ("th1x0_10", th1x0_10), ("th01", th01),
                          ("th2x0_01", th2x0_01), ("th11", th11),
                          ("th1x0_11", th1x0_11), ("th2x0_11", th2x0_11)):
                ths[nm] = pp.tile([U, U], dt.float32, name=nm + "_t")
                nc.sync.dma_start(out=ths[nm][:], in_=t.ap())
            th00_t = pp.tile([U, U], dt.bfloat16)
            nc.sync.dma_start(out=th00_t[:], in_=th00b.ap())
            thb_t = pp.tile([U, 1], dt.float32)
            nc.sync.dma_start(out=thb_t[:], in_=thbT.ap())

            # ---------------- phases A+B interleaved ------------------------
            # Row/t1 streams first -> rt table + collectives early, so the
            # Pool engine can start phase-C gathers while the t0-col stream
            # and ct table still run on PE/DVE.
            ct_stage = pp.tile([128, NWc, U], dt.float32)
            ct_stageb = pp.tile([128, NWc, U], dt.bfloat16)

            rt_bf_slice = dram.tile([NP, 128], dt.bfloat16)
            rt_bf_all = dram.tile([RT_ROWS, 128], dt.bfloat16,
                                  addr_space="Shared")

            with tc.tile_pool(name="pa", bufs=3) as pa, \
                 tc.tile_pool(name="poh", bufs=2) as poh, \
                 tc.tile_pool(name="pas", bufs=2, space="PSUM") as pas, \
                 tc.tile_pool(name="prel", bufs=1) as prel, \
                 tc.tile_pool(name="pb", bufs=2) as pb, \
                 tc.tile_pool(name="pbs", bufs=1, space="PSUM") as pbs:

                KC = 32  # slots per oh/x chunk

                def run_stream(si, xa, rel_d, K, NW, soff, S):
                    rel_t = prel.tile([128, S], dt.bfloat16, name=f"rel_t{si}",
                                      tag=f"rel{si}")
                    nc.sync.dma_start(out=rel_t[:], in_=rel_d.ap())
                    for w in range(NW):
                        pw = pas.tile([U, 128], dt.float32, space="PSUM", tag="pw")
                        for k0 in range(0, K, KC):
                            kc = min(KC, K - k0)
                            s0 = w * K + k0
                            xw = pa.tile([128, KC, U], dt.bfloat16, tag="xw")
                            nc.sync.dma_start(
                                out=xw[:, :kc, :],
                                in_=xa.ap()[s0:s0 + kc].rearrange("s p f -> p s f"))
                            # batched one-hot build: [128, kc, 128], one DVE op
                            oh = poh.tile([128, KC, 128], dt.bfloat16, tag="oh")
                            in0 = (rel_t[:, s0:s0 + kc]
                                   .rearrange("p (k f) -> p k f", f=1)
                                   .to_broadcast([128, kc, 128]))
                            in1 = (iota_t[:]
                                   .rearrange("p (k f) -> p k f", k=1)
                                   .to_broadcast([128, kc, 128]))
                            nc.vector.tensor_tensor(
                                out=oh[:, :kc, :], in0=in0, in1=in1,
                                op=mybir.AluOpType.is_equal)
                            for k in range(kc):
                                nc.tensor.matmul(
                                    pw[:], lhsT=xw[:, k, :], rhs=oh[:, k, :],
                                    start=(k0 + k == 0),
                                    stop=(k0 + k == K - 1))
                        nc.vector.tensor_copy(
                            out=sumT[:, soff + w * 128: soff + (w + 1) * 128],
                            in_=pw[:])

                invs = {}
                for nm, t, ln in (("inv_c0", inv_c0, MP), ("inv_r0", inv_r0, NP),
                                  ("inv_c1", inv_c1, MP), ("inv_r2", inv_r2, NP)):
                    invs[nm] = pab.tile([1, ln], dt.float32, name=nm + "_t")
                    nc.sync.dma_start(out=invs[nm][:], in_=t.ap())

                rt_stage = pab.tile([128, NWr, 128], dt.bfloat16)
                nc.vector.memset(rt_stage[:], 0.0)

                # --- early streams: everything the rt table + totals need ---
                run_stream(1, x0r_a, rel0r, K0r, NWr, off_r0, S0r)
                run_stream(2, x1c_a, rel1c, K1c, NWc, off_c1, S1c)
                run_stream(3, x2r_a, rel2r, K2r, NWr, off_r2, S2r)

                def table_chunk(ci, inv_a, inv_b, soff_a, soff_b, thA, thB,
                                add_g, is_ct):
                    sl = slice(ci * 128, (ci + 1) * 128)
                    pr = pbs.tile([U, 128], dt.float32, space="PSUM", tag="pr")
                    nc.tensor.matmul(pr[:], lhsT=ones_t[:], rhs=inv_a[:, sl],
                                     start=True, stop=True)
                    m0 = pb.tile([U, 128], dt.float32, tag="m0")
                    nc.vector.tensor_mul(out=m0[:],
                                         in0=sumT[:, soff_a + ci * 128:
                                                  soff_a + (ci + 1) * 128],
                                         in1=pr[:])
                    pr2 = pbs.tile([U, 128], dt.float32, space="PSUM", tag="pr2")
                    nc.tensor.matmul(pr2[:], lhsT=ones_t[:], rhs=inv_b[:, sl],
                                     start=True, stop=True)
                    m1 = pb.tile([U, 128], dt.float32, tag="m1")
                    nc.vector.tensor_mul(out=m1[:],
                                         in0=sumT[:, soff_b + ci * 128:
                                                  soff_b + (ci + 1) * 128],
                                         in1=pr2[:])
                    pc = pbs.tile([U, 128], dt.float32, space="PSUM", tag="pc")
                    nc.tensor.matmul(pc[:], lhsT=thA[:], rhs=m0[:],
                                     start=True, stop=False)
                    nc.tensor.matmul(pc[:], lhsT=thB[:], rhs=m1[:],
                                     start=False, stop=True)
                    cf = pb.tile([U, 128], dt.float32, tag="cf")
                    if add_g:
                        nc.vector.tensor_tensor(
                            out=cf[:], in0=pc[:],
                            in1=g_t[:].to_broadcast([U, 128]),
                            op=mybir.AluOpType.add)
                    else:
                        nc.vector.tensor_copy(out=cf[:], in_=pc[:])
                    pt = pbs.tile([128, U], dt.float32, space="PSUM", tag="pt")
                    nc.tensor.transpose(out=pt[:], in_=cf[:],
                                        identity=ident_t[:U, :U])
                    if is_ct:
                        nc.vector.tensor_copy(out=ct_stage[:, ci, :], in_=pt[:])
                        nc.vector.tensor_copy(out=ct_stageb[:, ci, :], in_=pt[:])
                    else:
                        nc.vector.tensor_copy(out=rt_stage[:, ci, :U], in_=pt[:])

                # rt table first, then AllGather -> unblocks phase-C gathers
                for ci in range(NWr):
                    table_chunk(ci, invs["inv_r0"], invs["inv_r2"], off_r0,
                                off_r2, ths["th01"], ths["th2x0_01"], False,
                                False)
                nc.sync.dma_start(
                    out=rt_bf_slice[0:NP].rearrange("(c p) f -> p c f", p=128),
                    in_=rt_stage[:])
                nc.gpsimd.collective_compute(
                    "AllGather", mybir.AluOpType.bypass,
                    ins=[rt_bf_slice.opt()], outs=[rt_bf_all.opt()],
                    replica_groups=[list(range(NCORES))])

                # grand totals AFTER the AllGather on the pool queue, so the
                # phase-C gathers are only gated by the AllGather itself.
                # t0 total via ROW sums (same value), t1 col, t2 row.
                totL = pp.tile([U, 4], dt.float32)
                nc.vector.memset(totL[:], 0.0)
                nc.vector.tensor_reduce(
                    out=totL[:, 0:1], in_=sumT[:, off_r0:off_r0 + NP],
                    axis=mybir.AxisListType.X, op=mybir.AluOpType.add)
                nc.vector.tensor_reduce(
                    out=totL[:, 1:2], in_=sumT[:, off_c1:off_c1 + MP],
                    axis=mybir.AxisListType.X, op=mybir.AluOpType.add)
                nc.vector.tensor_reduce(
                    out=totL[:, 2:3], in_=sumT[:, off_r2:off_r2 + NP],
                    axis=mybir.AxisListType.X, op=mybir.AluOpType.add)

                totb = dram.tile([U, 4], dt.float32)
                totg = dram.tile([U, 4], dt.float32, addr_space="Shared")
                nc.gpsimd.dma_start(out=totb[:], in_=totL[:])
                nc.gpsimd.collective_compute(
                    "AllReduce", mybir.AluOpType.add,
                    ins=[totb.opt()], outs=[totg.opt()],
                    replica_groups=[list(range(NCORES))])
                totG = pp.tile([U, 4], dt.float32)
                nc.gpsimd.dma_start(out=totG[:], in_=totg[:])

                # --- late: t0 col stream + global term + ct table ----------
                run_stream(0, x0c_a, rel0c, K0c, NWc, off_c0, S0c)

                mv = pp.tile([U, 4], dt.float32)
                for j, nnz in ((0, meta["NNZ0"]), (1, meta["NNZ1"]),
                               (2, meta["NNZ2"])):
                    nc.vector.tensor_scalar_mul(
                        out=mv[:, j:j + 1], in0=totG[:, j:j + 1],
                        scalar1=float(1.0 / nnz))
                gp = pbs.tile([U, 1], dt.float32, space="PSUM", tag="gp")
                nc.tensor.matmul(gp[:], lhsT=ths["th11"][:], rhs=mv[:, 0:1],
                                 start=True, stop=False)
                nc.tensor.matmul(gp[:], lhsT=ths["th1x0_11"][:], rhs=mv[:, 1:2],
                                 start=False, stop=False)
                nc.tensor.matmul(gp[:], lhsT=ths["th2x0_11"][:], rhs=mv[:, 2:3],
                                 start=False, stop=True)
                g_t = pp.tile([U, 1], dt.float32)
                nc.vector.tensor_add(out=g_t[:], in0=gp[:], in1=thb_t[:])

                for ci in range(NWc):
                    table_chunk(ci, invs["inv_c0"], invs["inv_c1"], off_c0,
                                off_c1, ths["th10"], ths["th1x0_10"], True, True)
                if debug:
                    nc.sync.dma_start(out=ct_dump.ap(), in_=ct_stage[:])
                    with tc.tile_pool(name="pdbg", bufs=2) as pdbg:
                        for b in range(RT_ROWS // 128):
                            dtile = pdbg.tile([128, 128], dt.bfloat16, tag="dt")
                            nc.sync.dma_start(
                                out=dtile[:],
                                in_=rt_bf_all[b * 128:(b + 1) * 128])
                            dtf = pdbg.tile([128, 128], dt.float32, tag="dtf")
                            nc.vector.tensor_copy(out=dtf[:], in_=dtile[:])
                            nc.sync.dma_start(
                                out=rtt_dump.ap()[b * 128:(b + 1) * 128],
                                in_=dtf[:])

            pab_cm.__exit__(None, None, None)

            # ---------------- phase C: per-entry output (transposed) --------
            with tc.tile_pool(name="pcx", bufs=3) as pcx, \
                 tc.tile_pool(name="pcg", bufs=3) as pcg, \
                 tc.tile_pool(name="pcr", bufs=2) as pcr, \
                 tc.tile_pool(name="pco", bufs=2) as pco, \
                 tc.tile_pool(name="pct", bufs=3) as pct, \
                 tc.tile_pool(name="pcs", bufs=3, space="PSUM") as pcs, \
                 tc.tile_pool(name="pcs2", bufs=2, space="PSUM") as pcs2, \
                 tc.tile_pool(name="ppos", bufs=1) as ppos:

                rpos_t = ppos.tile([128, 2 * NG * (NIDX // 16)], dt.int16)
                nc.sync.dma_start(out=rpos_t[:], in_=rpos.ap())

                gather_src = rt_bf_all[BIAS:] if BIAS else rt_bf_all[0:]

                K = K0c
                NW16 = NIDX // 16
                for g in range(NG):
                    # rt rows, entry-major bf16; two half-gathers on two
                    # queues so their desc-gen overlaps on different Q7 pairs
                    rtg2 = []
                    for h in range(2):
                        rtg = pcg.tile([128, NIDX // 128, 128], dt.bfloat16,
                                       tag=f"rtg{h}")
                        hi = 2 * g + h
                        nc.gpsimd.dma_gather(
                            out_ap=rtg[:], in_ap=gather_src,
                            idxs_ap=rpos_t[:, hi * NW16:(hi + 1) * NW16],
                            num_idxs=NIDX, num_idxs_reg=NIDX, elem_size=128,
                            single_packet=False,
                            queue_num=hi % 4)
                        rtg2.append(rtg)
                    # x stream chunk [64, GM]
                    xc = pcx.tile([U, GM], dt.bfloat16, tag="xc")
                    nc.sync.dma_start(out=xc[:], in_=xT.ap()[:, g * GM:(g + 1) * GM])
                    # rel chunk [1, GM]
                    rc = pcr.tile([1, GM], dt.bfloat16, tag="rc")
                    nc.sync.dma_start(out=rc[:], in_=relC.ap()[:, g * GM:(g + 1) * GM])
                    ot = pco.tile([U, GM], dt.float32, tag="ot")
                    for j in range(GM // CH):
                        e0 = g * GM + j * CH
                        # replicate rel across partitions: K=1 ones matmul
                        rp = pcs2.tile([128, CH], dt.float32, space="PSUM",
                                       tag="rp")
                        nc.tensor.matmul(rp[:], lhsT=onesr_t[:],
                                         rhs=rc[:, j * CH:(j + 1) * CH],
                                         start=True, stop=True)
                        ohT = pct.tile([128, CH], dt.bfloat16, tag="ohT")
                        nc.vector.tensor_tensor(
                            out=ohT[:], in0=rp[:],
                            in1=iotaP_t[:].to_broadcast([128, CH]),
                            op=mybir.AluOpType.is_equal)
                        if debug and g == 0 and j == 0:
                            ohTd = pct.tile([128, CH], dt.float32, tag="ohTd")
                            nc.vector.tensor_copy(out=ohTd[:], in_=ohT[:])
                            nc.sync.dma_start(out=oht_dump.ap(), in_=ohTd[:])
                        # z + ct + rt accumulation
                        acc = pcs.tile([U, CH], dt.float32, space="PSUM",
                                       tag="acc")
                        nc.tensor.matmul(acc[:], lhsT=th00_t[:],
                                         rhs=xc[:, j * CH:(j + 1) * CH],
                                         start=True, stop=False)
                        for q in range(CH // 128):
                            s = (e0 + q * 128) // 128  # global slot
                            w = min(s // K, NWc - 1)   # pad slots: ohT is 0
                            nc.tensor.matmul(
                                acc[:, q * 128:(q + 1) * 128],
                                lhsT=ct_stageb[:, w, :],
                                rhs=ohT[:, q * 128:(q + 1) * 128],
                                start=False, stop=False)
                            # rt rows: transpose gathered slot via identity
                            sm = (j * CH) // 128 + q   # slot in macro [0,32)
                            HS = GM // 256             # real slots per half
                            nc.tensor.matmul(
                                acc[:, q * 128:(q + 1) * 128],
                                lhsT=rtg2[sm // HS][:, sm % HS, 0:U],
                                rhs=identb_t[:],
                                start=False, stop=True)
                        nc.scalar.activation(
                            out=ot[:, j * CH:(j + 1) * CH], in_=acc[:],
                            func=mybir.ActivationFunctionType.Relu)
                    nc.sync.dma_start(
                        out=out_d.ap()[:, g * GM:(g + 1) * GM], in_=ot[:])

    nc.compile()
    _PROG_CACHE[key] = nc
    return nc


# --------------------------------------------------------------------------
# entry point
# --------------------------------------------------------------------------

def _run(inputs, dims, trace=False, debug=False):
    meta, in_maps, post = _prepare(inputs, dims)
    nc = _build_program(meta, debug=debug)
    res = run_bass_kernel_spmd(nc, in_maps, core_ids=list(range(NCORES)),
                               trace=trace)
    NNZ0 = dims["NNZ0"]
    out = np.empty((NNZ0, U), np.float32)
    for c in range(NCORES):
        o = res.results[c]["out_d"]            # [64, E]
        p = post[c]
        if p["clen"] == 0:
            continue
        out[p["corder"]] = o[:, p["epos"]].T
    return out, res


def kernel(**inputs):
    out, _ = _run(inputs, FULL_DIMS, trace=False)
    return out


# ------- helpers for test harness ------------------------------------------

def install_ntff_hook():
    """Enable NTFF profiling under axon (exec_time_ns in results)."""
    try:
        import antenv
        mod = types.ModuleType("antenv.axon_hooks")
        _h = [None]
        mod.set_axon_ntff_profile_hook = lambda h: _h.__setitem__(0, h)
        mod.get_axon_ntff_profile_hook = lambda: _h[0]
        sys.modules["antenv.axon_hooks"] = mod
        antenv.axon_hooks = mod
        from trn_agent_boot.trn_boot import _ntff_profile_via_ctypes
        mod.set_axon_ntff_profile_hook(
            _ntff_profile_via_ctypes("/opt/axon/libaxon_pjrt.so"))
        return True
    except Exception as e:  # pragma: no cover
        print("ntff hook install failed:", e)
        return False


def ref_numpy(inputs, dims):
    """Numpy port of the reference (for arbitrary dims)."""
    N, M = dims["N"], dims["M"]
    x0 = np.asarray(inputs["t0_values"], np.float64)
    x1 = np.asarray(inputs["t1_values"], np.float64)
    x2 = np.asarray(inputs["t2_values"], np.float64)
    tr = np.asarray(inputs["t0_rows"]); tcl = np.asarray(inputs["t0_cols"])
    t1c = np.asarray(inputs["t1_cols"]); t2r = np.asarray(inputs["t2_rows"])

    def segmean(v, ids, n):
        s = np.zeros((n, v.shape[1])); np.add.at(s, ids, v)
        c = np.bincount(ids, minlength=n).astype(np.float64)
        return s / (c + EPS)[:, None]

    th = {k: np.asarray(inputs[k], np.float64) for k in
          ("theta_00", "theta_10", "theta_01", "theta_11", "theta_1x0_10",
           "theta_1x0_11", "theta_2x0_01", "theta_2x0_11")}
    vals = x0 @ th["theta_00"]
    vals += (segmean(x0, tcl, M) @ th["theta_10"])[tcl]
    vals += (segmean(x0, tr, N) @ th["theta_01"])[tr]
    vals += x0.mean(0) @ th["theta_11"]
    vals += (segmean(x1, t1c, M) @ th["theta_1x0_10"])[tcl]
    vals += x1.mean(0) @ th["theta_1x0_11"]
    vals += (segmean(x2, t2r, N) @ th["theta_2x0_01"])[tr]
    vals += x2.mean(0) @ th["theta_2x0_11"]
    vals += np.asarray(inputs["theta_b"], np.float64)
    return np.maximum(vals, 0.0).astype(np.float32)
